# revision 1
# baseline (speedup 1.0000x reference)
"""Bidirectional cross-attention kernel for 8 Trainium2 NeuronCores.

Sharding: 16 (batch, head) units across 8 cores -> core c handles
batch b = c//4 and heads (2*(c%4), 2*(c%4)+1).  Each core computes, for its
two heads:
    E   = exp(scale * qk @ m_qk^T)           (unnormalized, shared both ways)
    M1T = [v | 1]^T @ E                       -> m-side out^T + colsum row
    O1T = [m_v | 1]^T @ E^T                   -> x-side out^T + rowsum row
    px  = sum_h (O1T_h / rowsum)^T @ Wof_h    (Wof = Wo @ Wf folded on host)
    pm  = sum_h (M1T_h / colsum)^T @ mWof_h
Host sums the 4 per-batch partials and adds the folded biases.
E^T is produced by a DRAM round-trip through the DMA xbar transpose
(SBUF-source xbar transpose is not supported on this hardware).
"""

import numpy as np
import ml_dtypes

import concourse.bass as bass
import concourse.mybir as mybir
import concourse.tile as tile
from concourse import bacc
from concourse.bass_utils import run_bass_kernel_spmd
from concourse.masks import make_identity

F32 = mybir.dt.float32
BF16 = mybir.dt.bfloat16
EXP = mybir.ActivationFunctionType.Exp

N = 2048          # sequence length (i and j)
DM = 256          # model dim
DH = 64           # head dim
NT = N // 128     # 16 row tiles
SCALE = DH ** -0.5

_cache = {}
CFG = {"et": True, "edram": True, "m1t": True, "sim": True, "proj": True}


def _build():
    nc = bacc.Bacc("TRN2", target_bir_lowering=False, debug=False, num_devices=8)

    xl = nc.dram_tensor("xl", [N, DM], F32, kind="ExternalInput")
    ml = nc.dram_tensor("ml", [N, DM], F32, kind="ExternalInput")
    wqk = nc.dram_tensor("wqk", [DM, 128], F32, kind="ExternalInput")
    mwqk = nc.dram_tensor("mwqk", [DM, 128], F32, kind="ExternalInput")
    wv = nc.dram_tensor("wv", [DM, 128], F32, kind="ExternalInput")
    mwv = nc.dram_tensor("mwv", [DM, 128], F32, kind="ExternalInput")
    wof = nc.dram_tensor("wof", [128, DM], BF16, kind="ExternalInput")
    mwof = nc.dram_tensor("mwof", [128, DM], BF16, kind="ExternalInput")
    px = nc.dram_tensor("px", [N, DM], F32, kind="ExternalOutput")
    pm = nc.dram_tensor("pm", [N, DM], F32, kind="ExternalOutput")

    with tile.TileContext(nc) as tc:
        _emit(tc, xl, ml, wqk, mwqk, wv, mwv, wof, mwof, px, pm)
    nc.compile()
    return nc


def _emit(tc, xl, ml, wqk, mwqk, wv, mwv, wof, mwof, px, pm):
    nc = tc.nc
    import contextlib
    ctx = contextlib.ExitStack()
    with ctx:
        singles = ctx.enter_context(tc.tile_pool(name="singles", bufs=1))
        xin_p = ctx.enter_context(tc.tile_pool(name="xin", bufs=5))
        rec_p = ctx.enter_context(tc.tile_pool(name="rec", bufs=2))
        e_p = ctx.enter_context(tc.tile_pool(name="et", bufs=4))
        et_p = ctx.enter_context(tc.tile_pool(name="ett", bufs=16))
        big_p = ctx.enter_context(tc.tile_pool(name="big", bufs=2))
        bc_p = ctx.enter_context(tc.tile_pool(name="bc", bufs=1))
        out_p = ctx.enter_context(tc.tile_pool(name="outp", bufs=2))
        psim_p = ctx.enter_context(tc.tile_pool(name="psim", bufs=2, space="PSUM"))
        pacc_p = ctx.enter_context(tc.tile_pool(name="pacc", bufs=4, space="PSUM"))
        dram_p = ctx.enter_context(tc.tile_pool(name="dram", bufs=2, space="DRAM"))

        ident = singles.tile([128, 128], F32)
        make_identity(nc, ident)

        # ---- load weights ----
        wqk_sb = singles.tile([128, 2, 128], F32)
        mwqk_sb = singles.tile([128, 2, 128], F32)
        wv_sb = singles.tile([128, 2, 128], F32)
        mwv_sb = singles.tile([128, 2, 128], F32)
        for t, d in ((wqk_sb, wqk), (mwqk_sb, mwqk), (wv_sb, wv), (mwv_sb, mwv)):
            nc.sync.dma_start(out=t[:], in_=d.rearrange("(k p) n -> p k n", p=128))
        wof_sb = singles.tile([64, 2, DM], BF16)
        mwof_sb = singles.tile([64, 2, DM], BF16)
        nc.sync.dma_start(out=wof_sb[:], in_=wof.rearrange("(h d) n -> d h n", d=64))
        nc.sync.dma_start(out=mwof_sb[:], in_=mwof.rearrange("(h d) n -> d h n", d=64))

        # ---- transpose x, m into [dm, n] layout ----
        xT = singles.tile([128, 2, N], F32)   # [dm%128, dm//128, n]
        mT = singles.tile([128, 2, N], F32)
        for src, dst in ((xl, xT), (ml, mT)):
            for tg in range(NT // 4):
                xins = []
                for q in range(4):
                    xin = xin_p.tile([128, DM], F32, tag="xin")
                    t = tg * 4 + q
                    nc.sync.dma_start(out=xin[:], in_=src[t * 128:(t + 1) * 128, :])
                    xins.append(xin)
                for kc in range(2):
                    pt = psim_p.tile([128, 512], F32, tag="psim")
                    for q in range(4):
                        nc.tensor.transpose(pt[:, q * 128:(q + 1) * 128],
                                            xins[q][:, kc * 128:(kc + 1) * 128], ident)
                    nc.vector.tensor_copy(
                        dst[:, kc, tg * 512:(tg + 1) * 512], pt[:])

        # ---- projections ----
        qkT = singles.tile([128, N], F32)     # rows 0:64 head0, 64:128 head1
        m_qkT = singles.tile([128, N], F32)
        for w_sb, src, dst in ((wqk_sb, xT, qkT), (mwqk_sb, mT, m_qkT)):
            for nt in range(4):
                pq = pacc_p.tile([128, 512], F32, tag="pacc")
                for kc in range(2):
                    nc.tensor.matmul(pq[:], w_sb[:, kc, :],
                                     src[:, kc, nt * 512:(nt + 1) * 512],
                                     start=(kc == 0), stop=(kc == 1))
                eng = nc.vector.tensor_copy
                eng(dst[:, nt * 512:(nt + 1) * 512], pq[:])

        # v / m_v in natural layout with ones column: [128, t, head, 65]
        v_sb = singles.tile([128, NT, 2, 65], BF16)
        mv_sb = singles.tile([128, NT, 2, 65], BF16)
        for w_sb, src, dst in ((wv_sb, xT, v_sb), (mwv_sb, mT, mv_sb)):
            for t in range(NT):
                pv = pacc_p.tile([128, 128], F32, tag="pacc")
                for kc in range(2):
                    nc.tensor.matmul(pv[:], src[:, kc, t * 128:(t + 1) * 128],
                                     w_sb[:, kc, :], start=(kc == 0), stop=(kc == 1))
                eng = nc.vector.tensor_copy
                eng(dst[:, t, :, 0:64], pv.rearrange("p (h d) -> p h d", h=2))
            nc.vector.memset(dst[:, :, :, 64:65], 1.0)

        outT_b = singles.tile([64, 2, N], BF16)    # x-side normalized out^T
        m_outT_b = singles.tile([64, 2, N], BF16)  # m-side normalized out^T

        Edrams = [[nc.dram_tensor(f"edram{h}_{hf}", [N // 2, N], BF16).ap()
                   for hf in range(2)] for h in range(2)]
        for h in range(2):
            Edram = Edrams[h]
            # ---- phase A: sim -> exp -> E, M1T accumulation, E -> DRAM ----
            pM1T = []
            for _jc in range(4 if CFG["m1t"] else 0):
                pt_m1 = pacc_p.tile([128, 512], F32, tag="pacc")
                pM1T.append(pt_m1)
            for ic in range(NT):
                Et = e_p.tile([128, N], BF16, tag="et")
                for half in range(2):
                    ps = psim_p.tile([128, 1024], F32, tag="psim")
                    for q in range(2 if CFG["sim"] else 0):
                        jn = half * 2 + q
                        nc.tensor.matmul(
                            ps[:, q * 512:(q + 1) * 512],
                            qkT[h * 64:(h + 1) * 64, ic * 128:(ic + 1) * 128],
                            m_qkT[h * 64:(h + 1) * 64, jn * 512:(jn + 1) * 512],
                            start=True, stop=True)
                    nc.scalar.activation(Et[:, half * 1024:(half + 1) * 1024],
                                         ps[:], EXP, scale=SCALE)
                if CFG["edram"]:
                    nc.gpsimd.dma_start(
                        out=Edram[ic // 8][(ic % 8) * 128:(ic % 8 + 1) * 128, :],
                        in_=Et[:])
                for jc in range(4 if CFG["m1t"] else 0):
                    nc.tensor.matmul(pM1T[jc][0:65, :], v_sb[:, ic, h, :],
                                     Et[:, jc * 512:(jc + 1) * 512],
                                     start=(ic == 0), stop=(ic == NT - 1))

            # ---- m-side normalize ----
            M1Tf = big_p.tile([65, N], F32, tag="acc_f32")
            if not CFG["m1t"]:
                nc.vector.memset(M1Tf[:], 1.0)
            for jc in range(4 if CFG["m1t"] else 0):
                eng = nc.vector.tensor_copy
                eng(M1Tf[:, jc * 512:(jc + 1) * 512], pM1T[jc][0:65, :])
            crec = rec_p.tile([1, N], F32, tag="rec")
            nc.vector.reciprocal(crec[:], M1Tf[64:65, :])
            crec_d = dram_p.tile([1, N], F32, tag="rec_d")
            nc.sync.dma_start(out=crec_d[:], in_=crec[:])
            cb = bc_p.tile([64, N], F32, tag="bcast")
            nc.gpsimd.dma_start(
                out=cb[:],
                in_=bass.AP(tensor=crec_d.tensor, offset=crec_d.offset,
                            ap=[[0, 64], [1, N]]))
            nc.vector.tensor_mul(m_outT_b[:, h, :], M1Tf[0:64, :], cb[:])

            # hoist head-0's E^T xbar reads so they overlap head-1's phase A
            if h == 0 and CFG["et"]:
                ETs0 = []
                for jt in range(NT):
                    et0 = et_p.tile([128, N], BF16, tag="ett")
                    for hf in range(2):
                        nc.sync.dma_start_transpose(
                            out=et0[:, hf * 1024:(hf + 1) * 1024],
                            in_=Edram[hf][:, jt * 128:(jt + 1) * 128])
                    ETs0.append(et0)

        for h in range(2):
            Edram = Edrams[h]
            # ---- x-side via DRAM xbar transpose of E, then O1T ----
            O1Tf = big_p.tile([65, N], F32, tag="acc_f32")
            if CFG["et"]:
                if h == 0:
                    ETs = ETs0
                else:
                    ETs = []
                    for jt in range(NT):
                        et = et_p.tile([128, N], BF16, tag="ett")
                        for hf in range(2):
                            nc.sync.dma_start_transpose(
                                out=et[:, hf * 1024:(hf + 1) * 1024],
                                in_=Edram[hf][:, jt * 128:(jt + 1) * 128])
                        ETs.append(et)
                for iN in range(4):
                    pO = pacc_p.tile([128, 512], F32, tag="pacc")
                    for jt in range(NT):
                        nc.tensor.matmul(pO[0:65, :], mv_sb[:, jt, h, :],
                                         ETs[jt][:, iN * 512:(iN + 1) * 512],
                                         start=(jt == 0), stop=(jt == NT - 1))
                    eng = nc.vector.tensor_copy
                    eng(O1Tf[:, iN * 512:(iN + 1) * 512], pO[0:65, :])
            elif CFG["et"]:
                pO1p = []
                for _iN in range(4):
                    pt_o1 = pacc_p.tile([128, 512], F32, tag="pacc")
                    pO1p.append(pt_o1)
                for jc in range(NT):
                    Ep = e_p.tile([128, N], BF16, tag="et")
                    for half in range(2):
                        ps = psim_p.tile([128, 1024], F32, tag="psim")
                        for q in range(2):
                            iN = half * 2 + q
                            nc.tensor.matmul(
                                ps[:, q * 512:(q + 1) * 512],
                                m_qkT[h * 64:(h + 1) * 64, jc * 128:(jc + 1) * 128],
                                qkT[h * 64:(h + 1) * 64, iN * 512:(iN + 1) * 512],
                                start=True, stop=True)
                        nc.scalar.activation(Ep[:, half * 1024:(half + 1) * 1024],
                                             ps[:], EXP, scale=SCALE)
                    for iN in range(4):
                        nc.tensor.matmul(pO1p[iN][0:65, :], mv_sb[:, jc, h, :],
                                         Ep[:, iN * 512:(iN + 1) * 512],
                                         start=(jc == 0), stop=(jc == NT - 1))
                for iN in range(4):
                    eng = nc.vector.tensor_copy
                    eng(O1Tf[:, iN * 512:(iN + 1) * 512], pO1p[iN][0:65, :])
            else:
                nc.vector.memset(O1Tf[:], 1.0)
            rrec = rec_p.tile([1, N], F32, tag="rec")
            nc.vector.reciprocal(rrec[:], O1Tf[64:65, :])
            rrec_d = dram_p.tile([1, N], F32, tag="rec_d")
            nc.sync.dma_start(out=rrec_d[:], in_=rrec[:])
            rb = bc_p.tile([64, N], F32, tag="bcast")
            nc.gpsimd.dma_start(
                out=rb[:],
                in_=bass.AP(tensor=rrec_d.tensor, offset=rrec_d.offset,
                            ap=[[0, 64], [1, N]]))
            nc.vector.tensor_mul(outT_b[:, h, :], O1Tf[0:64, :], rb[:])

        # ---- output projections: accumulate both heads ----
        for src, w_sb, dst in ((outT_b, wof_sb, px), (m_outT_b, mwof_sb, pm)):
            for t in range(NT):
                pP = psim_p.tile([128, DM], F32, tag="psim")
                nc.tensor.matmul(pP[:], src[:, 0, t * 128:(t + 1) * 128],
                                 w_sb[:, 0, :], start=True, stop=False)
                nc.tensor.matmul(pP[:], src[:, 1, t * 128:(t + 1) * 128],
                                 w_sb[:, 1, :], start=False, stop=True)
                po = out_p.tile([128, DM], F32, tag="outp")
                eng = nc.vector.tensor_copy
                eng(po[:], pP[:])
                nc.sync.dma_start(out=dst[t * 128:(t + 1) * 128, :], in_=po[:])


def kernel(x, m, Wqk, mWqk, Wv, mWv, Wo, bo, mWo, mbo, Wf, bf):
    x = np.asarray(x, dtype=np.float32)
    m = np.asarray(m, dtype=np.float32)
    Wqk = np.asarray(Wqk, dtype=np.float32)
    mWqk = np.asarray(mWqk, dtype=np.float32)
    Wv = np.asarray(Wv, dtype=np.float32)
    mWv = np.asarray(mWv, dtype=np.float32)
    Wo = np.asarray(Wo, dtype=np.float32)
    mWo = np.asarray(mWo, dtype=np.float32)
    Wf = np.asarray(Wf, dtype=np.float32)
    bo = np.asarray(bo, dtype=np.float32)
    mbo = np.asarray(mbo, dtype=np.float32)
    bf = np.asarray(bf, dtype=np.float32)

    if "nc" not in _cache:
        _cache["nc"] = _build()
    nc = _cache["nc"]

    wof = (Wo @ Wf).astype(ml_dtypes.bfloat16)    # [512, 256]
    mwof = (mWo @ Wf).astype(ml_dtypes.bfloat16)
    bias_x = bo @ Wf + bf
    bias_m = mbo @ Wf + bf

    in_maps = []
    for c in range(8):
        b, hp = c // 4, c % 4
        cs = slice(hp * 128, (hp + 1) * 128)
        in_maps.append({
            "xl": x[b], "ml": m[b],
            "wqk": np.ascontiguousarray(Wqk[:, cs]),
            "mwqk": np.ascontiguousarray(mWqk[:, cs]),
            "wv": np.ascontiguousarray(Wv[:, cs]),
            "mwv": np.ascontiguousarray(mWv[:, cs]),
            "wof": np.ascontiguousarray(wof[cs, :]),
            "mwof": np.ascontiguousarray(mwof[cs, :]),
        })

    res = run_bass_kernel_spmd(nc, in_maps, list(range(8)))

    out = np.empty((2, 2 * N, DM), dtype=np.float32)
    for b in range(2):
        cores = range(b * 4, b * 4 + 4)
        out[b, :N] = sum(res.results[c]["px"] for c in cores) + bias_x
        out[b, N:] = sum(res.results[c]["pm"] for c in cores) + bias_m
    return out



# revision 55
# speedup vs baseline: 2.1302x; 2.1302x over previous
"""Bidirectional cross-attention kernel for 8 Trainium2 NeuronCores.

Sharding: 16 (batch, head) units across 8 cores -> core c handles
batch b = c//4 and heads (2*(c%4), 2*(c%4)+1).

Per head on each core (all matmul data bf16):
    E   = exp(scale * qk @ m_qk^T)      [i, j] natural, exp on Act engine
          (exp's accum_out gives rowsum[i] for free)
    m-side: m_out[j, d] = sum_i E[i,j] v1[i,d]  via E tiles as stationary
            lhsT (out free = 64 -> half streaming cost); 65th ones column of
            v1 handled by separate N=1 colsum matmuls.
    x-side: needs E^T [j, i]: NA/16 of j-columns round-trip through DRAM and
            the DMA xbar transpose (14 ns per 16x128 tile in the cost model);
            the rest are recomputed as exp(sim^T) on chip.
            out[i, d] = sum_j E^T[j,i]^T mv[j,d] with E^T slabs stationary.
    Normalization = per-partition scalar multiply (DVE tensor_scalar) by
    1/rowsum / 1/colsum during the PSUM->SBUF drain, then PE transposes to
    get out^T for the folded output projection Wof = Wo @ Wf (host-folded).
Host sums the 4 per-batch partials and adds the folded biases.
"""

import numpy as np
import ml_dtypes

import concourse.bass as bass
import concourse.mybir as mybir
import concourse.tile as tile
from concourse import bacc
from concourse.bass_utils import run_bass_kernel_spmd
from concourse.masks import make_identity

F32 = mybir.dt.float32
BF16 = mybir.dt.bfloat16
EXP = mybir.ActivationFunctionType.Exp

import os

N = 2048          # sequence length (i and j)
DM = 256          # model dim
DH = 64           # head dim
NT = N // 128     # 16 row tiles
SCALE = DH ** -0.5
NA = int(os.environ.get("BXA_NA", "11"))   # E^T j-slabs via DRAM xbar
NG = NT - NA      # j-slabs of E^T recomputed on-chip (sim^T + exp)

_cache = {}


def _build():
    nc = bacc.Bacc("TRN2", target_bir_lowering=False, debug=False, num_devices=8)

    xT = nc.dram_tensor("xT", [DM, N], BF16, kind="ExternalInput")
    mT = nc.dram_tensor("mT", [DM, N], BF16, kind="ExternalInput")
    wqk = nc.dram_tensor("wqk", [DM, 128], BF16, kind="ExternalInput")
    mwqk = nc.dram_tensor("mwqk", [DM, 128], BF16, kind="ExternalInput")
    wv = nc.dram_tensor("wv", [DM, 128], BF16, kind="ExternalInput")
    mwv = nc.dram_tensor("mwv", [DM, 128], BF16, kind="ExternalInput")
    wof = nc.dram_tensor("wof", [128, DM], BF16, kind="ExternalInput")
    mwof = nc.dram_tensor("mwof", [128, DM], BF16, kind="ExternalInput")
    px = nc.dram_tensor("px", [N, DM], F32, kind="ExternalOutput")
    pm = nc.dram_tensor("pm", [N, DM], F32, kind="ExternalOutput")

    with tile.TileContext(nc) as tc:
        _emit(tc, xT, mT, wqk, mwqk, wv, mwv, wof, mwof, px, pm)
    nc.compile()
    return nc


def _emit(tc, xT, mT, wqk, mwqk, wv, mwv, wof, mwof, px, pm):
    nc = tc.nc
    import contextlib
    ctx = contextlib.ExitStack()
    with ctx:
        singles = ctx.enter_context(tc.tile_pool(name="singles", bufs=1))
        e_p = ctx.enter_context(tc.tile_pool(name="e", bufs=18))
        et_a = ctx.enter_context(tc.tile_pool(name="eta", bufs=max(NA, 1)))
        et_g = ctx.enter_context(tc.tile_pool(name="etg", bufs=max(NG, 1)))
        nrm_p = ctx.enter_context(tc.tile_pool(name="nrm", bufs=2))
        psim_p = ctx.enter_context(tc.tile_pool(name="psim", bufs=2, space="PSUM"))
        pacc_p = ctx.enter_context(tc.tile_pool(name="pacc", bufs=1, space="PSUM"))

        ident = singles.tile([128, 128], F32)
        make_identity(nc, ident)

        # ---- weights / inputs (qk weights + mT + xT first: they gate A0) ----
        wqk_sb = singles.tile([128, 2, 128], BF16)
        mwqk_sb = singles.tile([128, 2, 128], BF16)
        wv_sb = singles.tile([128, 2, 128], BF16)
        mwv_sb = singles.tile([128, 2, 128], BF16)
        for t, d in ((wqk_sb, wqk), (mwqk_sb, mwqk)):
            nc.sync.dma_start(out=t[:], in_=d.rearrange("(k p) n -> p k n", p=128))
        xT_sb = singles.tile([128, 2, N], BF16)
        mT_sb = singles.tile([128, 2, N], BF16)
        for src, dst, q in ((mT, mT_sb, 0), (mT, mT_sb, 1), (mT, mT_sb, 2),
                            (mT, mT_sb, 3), (xT, xT_sb, 0), (xT, xT_sb, 1),
                            (xT, xT_sb, 2), (xT, xT_sb, 3)):
            nc.sync.dma_start(
                out=dst[:, :, q * 512:(q + 1) * 512],
                in_=src.rearrange("(k p) n -> p k n", p=128)[:, :, q * 512:(q + 1) * 512])
        for t, d in ((wv_sb, wv), (mwv_sb, mwv)):
            nc.sync.dma_start(out=t[:], in_=d.rearrange("(k p) n -> p k n", p=128))
        wof_sb = singles.tile([64, 2, DM], BF16)
        mwof_sb = singles.tile([64, 2, DM], BF16)
        nc.sync.dma_start(out=wof_sb[:], in_=wof.rearrange("(h d) n -> d h n", d=64))
        nc.sync.dma_start(out=mwof_sb[:], in_=mwof.rearrange("(h d) n -> d h n", d=64))

        qkT = singles.tile([128, N], BF16)
        m_qkT = singles.tile([128, N], BF16)
        v_sb = singles.tile([128, NT, 2, 65], BF16)
        mv_sb = singles.tile([128, NT, 2, 65], BF16)
        rrecT = singles.tile([128, 2, NT], F32)       # 1/rowsum, i on partitions
        crecT = singles.tile([128, 2, NT], F32)       # 1/colsum, j on partitions
        outT_b = singles.tile([64, 2, N], BF16)       # x-side out^T
        m_outT_b = singles.tile([64, 2, N], BF16)     # m-side out^T
        po_m = singles.tile([128, 8, DM], F32)        # output staging (shared)
        po_x = po_m

        m_acc = pacc_p.tile([128, 1024], F32, tag="macc")   # banks 4-5
        x_acc = pacc_p.tile([128, 1024], F32, tag="xacc")   # banks 6-7
        cs = x_acc[:, 0:16]                                 # colsum sliver

        ESPLIT = int(os.environ.get("BXA_ESPLIT", "1"))
        Edram = [[nc.dram_tensor(f"edram{h}_{hf}", [N // 2, NA * 128], BF16).ap()
                  for hf in range(2)] for h in range(2)] if NA else [None, None]

        # PE p-state warmup: dummy transposes while the input DMAs stream in,
        # so the projections and first sims run at full clock
        for _ in range(14):
            nc.tensor.transpose(x_acc[:, 768:896], ident, ident)

        # ---- projections (psum borrowed from m_acc / x_acc halves) ----
        for wi, (w_sb, src, dst) in enumerate(
                ((mwqk_sb, mT_sb, m_qkT), (wqk_sb, xT_sb, qkT))):
            for t in range(4):
                s = (t % 2) * 512
                pq = (m_acc if wi == 0 else x_acc)[:, s:s + 512]
                for kc in range(2):
                    nc.tensor.matmul(pq, w_sb[:, kc, :],
                                     src[:, kc, t * 512:(t + 1) * 512],
                                     start=(kc == 0), stop=(kc == 1))
                nc.vector.tensor_copy(dst[:, t * 512:(t + 1) * 512], pq)
        for wi, (w_sb, src, dst) in enumerate(
                ((wv_sb, xT_sb, v_sb), (mwv_sb, mT_sb, mv_sb))):
            for g in range(4):
                k4 = (wi * 4 + g) % 4
                pv = (m_acc if k4 < 2 else x_acc)[:, (k4 % 2) * 512:
                                                  (k4 % 2) * 512 + 512]
                for dt_ in range(4):
                    t = g * 4 + dt_
                    for kc in range(2):
                        nc.tensor.matmul(pv[:, dt_ * 128:(dt_ + 1) * 128],
                                         src[:, kc, t * 128:(t + 1) * 128],
                                         w_sb[:, kc, :],
                                         start=(kc == 0), stop=(kc == 1))
                nc.vector.tensor_copy(
                    dst[:, g * 4:(g + 1) * 4, :, 0:64],
                    pv.rearrange("p (t h d) -> p t h d", t=4, h=2))
            nc.vector.memset(dst[:, :, :, 64:65], 1.0)

        Et = [[None] * NT, [None] * NT]
        ET = [[None] * NT, [None] * NT]

        # PSUM start=True zeroes the WHOLE 2KB bank (zero region), so a bank
        # shared by interleaved accumulation chains must see exactly ONE
        # start (its very first matmul) and ONE stop (its very last).
        # m-side chunk: for one ic, accumulate all 16 j-tiles (+ colsum for
        # h=0, where x_acc is free during phase A)
        def m_chunk(h, ic, with_cs):
            et_t = Et[h][ic]
            for jc in range(NT):
                nc.tensor.matmul(m_acc[:, jc * 64:(jc + 1) * 64],
                                 et_t[:, jc * 128:(jc + 1) * 128],
                                 v_sb[:, ic, h, 0:64],
                                 start=(ic == 0 and jc % 8 == 0),
                                 stop=(ic == NT - 1 and jc % 8 == 7),
                                 skip_group_check=True)
                if with_cs:
                    nc.tensor.matmul(cs[:, jc:jc + 1],
                                     et_t[:, jc * 128:(jc + 1) * 128],
                                     v_sb[:, ic, h, 64:65],
                                     start=(ic == 0 and jc == 0),
                                     stop=(ic == NT - 1 and jc == NT - 1),
                                     skip_group_check=True)

        # x-side chunk: one j-slab of E^T against all 16 i-tiles
        def x_chunk(h, jt, first, last):
            et_t = ET[h][jt]
            for ic in range(NT):
                nc.tensor.matmul(x_acc[:, ic * 64:(ic + 1) * 64],
                                 et_t[:, ic * 128:(ic + 1) * 128],
                                 mv_sb[:, jt, h, 0:64],
                                 start=(first and ic % 8 == 0),
                                 stop=(last and ic % 8 == 7),
                                 skip_group_check=True)

        def cs_burst(h):
            for ic in range(NT):
                et_t = Et[h][ic]
                for jc in range(NT):
                    nc.tensor.matmul(cs[:, jc:jc + 1],
                                     et_t[:, jc * 128:(jc + 1) * 128],
                                     v_sb[:, ic, h, 64:65],
                                     start=(ic == 0 and jc == 0),
                                     stop=(ic == NT - 1 and jc == NT - 1),
                                     skip_group_check=True)

        # rowsum via N=1 matmuls over the E^T slabs, into a borrowed psim tile
        def rs_burst(h):
            rs_t = psim_p.tile([128, 1024], F32, tag="psim", name=f"rs{h}")
            rs = rs_t[:, 0:16]
            for jt in range(NT):
                et_t = ET[h][jt]
                for ic in range(NT):
                    nc.tensor.matmul(rs[:, ic:ic + 1],
                                     et_t[:, ic * 128:(ic + 1) * 128],
                                     mv_sb[:, jt, h, 64:65],
                                     start=(jt == 0 and ic == 0),
                                     stop=(jt == NT - 1 and ic == NT - 1),
                                     skip_group_check=True)
            nc.vector.reciprocal(rrecT[:, h, :], rs)

        def phase_A(h, fillers):
            fi = 0
            for ic in range(NT):
                et_t = e_p.tile([128, N], BF16, tag="et", name=f"et{h}_{ic}")
                Et[h][ic] = et_t
                for half in range(2):
                    ps = psim_p.tile([128, 1024], F32, tag="psim", name="ps")
                    for q in range(2):
                        jn = half * 2 + q
                        nc.tensor.matmul(ps[:, q * 512:(q + 1) * 512],
                                         qkT[h * 64:(h + 1) * 64,
                                             ic * 128:(ic + 1) * 128],
                                         m_qkT[h * 64:(h + 1) * 64,
                                               jn * 512:(jn + 1) * 512],
                                         start=True, stop=True)
                    nc.scalar.activation(et_t[:, half * 1024:(half + 1) * 1024],
                                         ps[:], EXP, scale=SCALE)
                if NA:
                    nc.gpsimd.dma_start(
                        out=Edram[h][ic // 8][(ic % 8) * 128:(ic % 8 + 1) * 128, :],
                        in_=et_t[:, 0:NA * 128])
                while fi <= ic and fi < len(fillers):
                    fillers[fi]()
                    fi += 1
            while fi < len(fillers):
                fillers[fi]()
                fi += 1

        def alpha_grab(h):
            for jt in range(NA):
                et_t = et_a.tile([128, N], BF16, tag="eta", name=f"eta{h}_{jt}")
                ET[h][jt] = et_t

        def alpha_loads(h, ihalf):
            if not ESPLIT and ihalf == 1:
                return
            for jt in range(NA):
                if ESPLIT:
                    # split by i: first half gated only on stores ic<8
                    nc.sync.dma_start_transpose(
                        out=ET[h][jt][:, ihalf * 1024:(ihalf + 1) * 1024],
                        in_=Edram[h][ihalf][:, jt * 128:(jt + 1) * 128])
                else:
                    for hf in range(2):
                        nc.sync.dma_start_transpose(
                            out=ET[h][jt][:, hf * 1024:(hf + 1) * 1024],
                            in_=Edram[h][hf][:, jt * 128:(jt + 1) * 128])

        def gamma_slabs(h):
            for g in range(NG):
                jt = NA + g
                et_t = et_g.tile([128, N], BF16, tag="etg", name=f"etg{h}_{jt}")
                ET[h][jt] = et_t
                for half in range(2):
                    ps = psim_p.tile([128, 1024], F32, tag="psim", name="ps")
                    for q in range(2):
                        ich = half * 2 + q
                        nc.tensor.matmul(ps[:, q * 512:(q + 1) * 512],
                                         m_qkT[h * 64:(h + 1) * 64,
                                               jt * 128:(jt + 1) * 128],
                                         qkT[h * 64:(h + 1) * 64,
                                             ich * 512:(ich + 1) * 512],
                                         start=True, stop=True)
                    nc.scalar.activation(et_t[:, half * 1024:(half + 1) * 1024],
                                         ps[:], EXP, scale=SCALE)

        def colsum_rec(h):
            nc.vector.reciprocal(crecT[:, h, :], cs[:])

        # drain + transpose one side; acc is the psum accumulator being
        # drained, tp_base the (other) psum tensor lending [64, 512] regions
        def drain_transpose(h, acc, rec, dstT, tp_base, tag, act_share=False):
            xn = nrm_p.tile([128, 1024], F32, tag="xn", name=f"xn_{tag}{h}")
            for t in range(NT):
                if act_share and t % 2 == 1:
                    nc.scalar.mul(xn[:, t * 64:(t + 1) * 64],
                                  acc[:, t * 64:(t + 1) * 64],
                                  rec[:, h, t:t + 1])
                else:
                    nc.vector.tensor_scalar_mul(xn[:, t * 64:(t + 1) * 64],
                                                acc[:, t * 64:(t + 1) * 64],
                                                rec[:, h, t:t + 1])
            for b4 in range(4):
                tp = tp_base[0:64, (b4 % 2) * 512:(b4 % 2) * 512 + 512]
                for k in range(4):
                    t = b4 * 4 + k
                    nc.tensor.transpose(tp[:, k * 128:(k + 1) * 128],
                                        xn[:, t * 64:(t + 1) * 64], ident)
                if act_share and b4 % 2 == 1:
                    nc.scalar.mul(dstT[:, h, b4 * 512:(b4 + 1) * 512], tp, 1.0)
                else:
                    nc.vector.tensor_copy(dstT[:, h, b4 * 512:(b4 + 1) * 512], tp)

        def outproj(src, w_sb, dst_dram, pregs, dma_eng, po, act_share=True):
            # copies alternate DVE/Act to halve the serial chain; DMAs go in
            # groups of 4 tiles to amortize HWDGE issue cost
            dst_r = dst_dram.rearrange("(g t p) c -> p g t c", g=4, t=4, p=128)
            nr = len(pregs)
            for t in range(NT):
                reg = pregs[t % nr]
                nc.tensor.matmul(reg, src[:, 0, t * 128:(t + 1) * 128],
                                 w_sb[:, 0, :], start=True, stop=False)
                nc.tensor.matmul(reg, src[:, 1, t * 128:(t + 1) * 128],
                                 w_sb[:, 1, :], start=False, stop=True)
                if act_share and t % 2 == 1:
                    nc.scalar.mul(po[:, t % 8, :], reg, 1.0)
                else:
                    nc.vector.tensor_copy(po[:, t % 8, :], reg)
                if t % 4 == 3:
                    g = t // 4
                    nc_dma = dma_eng
                    nc_dma.dma_start(
                        out=dst_r[:, g, :, :],
                        in_=po[:, (g % 2) * 4:(g % 2) * 4 + 4, :])

        # ================= schedule =================
        # head 0 phase A; m-side + colsum interleaved (x_acc free), with a
        # 2-iteration stagger so chunks never wait on the exp just issued
        fillers0 = [lambda: None, lambda: None] + [
            (lambda ic=ic: m_chunk(0, ic, True)) for ic in range(NT)
        ]
        phase_A(0, fillers0)
        if NA:
            alpha_grab(0)
            alpha_loads(0, 0)   # first i-half gated only on stores ic<8
            alpha_loads(0, 1)
        colsum_rec(0)
        gamma_slabs(0)
        # m-side h0 drain + transposes (borrow x_acc halves for transposes)
        drain_transpose(0, m_acc, crecT, m_outT_b, x_acc, "m")

        # head 1 phase A; fillers: X0 gamma chunks + staggered m1 chunks
        xorder = list(range(NA, NT)) + list(range(NA))
        fillers1 = []
        for k in range(NT + 2):
            def f(k=k):
                if k < NG:
                    x_chunk(0, xorder[k], first=(k == 0),
                            last=(k == NT - 1))
                if k >= 2:
                    m_chunk(1, k - 2, False)
            fillers1.append(f)
        phase_A(1, fillers1)
        if NA:
            alpha_grab(1)
            alpha_loads(1, 0)
        # finish X0 with the alpha slabs (loaded during A1); rowsum burst must
        # precede gamma_slabs(1), whose exps reuse the slabs' pool slots
        for k in range(NG, NT):
            x_chunk(0, xorder[k], first=(k == 0), last=(k == NT - 1))
        rs_burst(0)
        gamma_slabs(1)
        if NA:
            alpha_loads(1, 1)

        # x-side h0 drain (x_acc frees up), then colsum burst h1 into cs
        xn0 = nrm_p.tile([128, 1024], F32, tag="xn", name="xn_x0")
        for t in range(NT):
            nc.vector.tensor_scalar_mul(xn0[:, t * 64:(t + 1) * 64],
                                        x_acc[:, t * 64:(t + 1) * 64],
                                        rrecT[:, 0, t:t + 1])
        cs_burst(1)
        colsum_rec(1)
        # x0 transposes into x_acc halves (free after cs read)
        for b4 in range(4):
            tp = x_acc[0:64, (b4 % 2) * 512:(b4 % 2) * 512 + 512]
            for k in range(4):
                t = b4 * 4 + k
                nc.tensor.transpose(tp[:, k * 128:(k + 1) * 128],
                                    xn0[:, t * 64:(t + 1) * 64], ident)
            nc.vector.tensor_copy(outT_b[:, 0, b4 * 512:(b4 + 1) * 512], tp)
        # m-side h1 drain + transposes into x_acc halves (x0 drained, X1 not
        # yet started) so pm-outproj's m_acc quarters stay conflict-free
        drain_transpose(1, m_acc, crecT, m_outT_b, x_acc, "m")
        mq = [m_acc[:, i * 256:(i + 1) * 256] for i in range(4)]
        xq = [x_acc[:, i * 256:(i + 1) * 256] for i in range(4)]
        outproj(m_outT_b, mwof_sb, pm, mq, nc.scalar, po_m)
        # x-side h1
        for k in range(NT):
            x_chunk(1, xorder[k], first=(k == 0), last=(k == NT - 1))
        rs_burst(1)
        drain_transpose(1, x_acc, rrecT, outT_b, m_acc, "x", act_share=True)
        outproj(outT_b, wof_sb, px, xq, nc.sync, po_x)


def kernel(x, m, Wqk, mWqk, Wv, mWv, Wo, bo, mWo, mbo, Wf, bf):
    x = np.asarray(x, dtype=np.float32)
    m = np.asarray(m, dtype=np.float32)
    Wqk = np.asarray(Wqk, dtype=np.float32)
    mWqk = np.asarray(mWqk, dtype=np.float32)
    Wv = np.asarray(Wv, dtype=np.float32)
    mWv = np.asarray(mWv, dtype=np.float32)
    Wo = np.asarray(Wo, dtype=np.float32)
    mWo = np.asarray(mWo, dtype=np.float32)
    Wf = np.asarray(Wf, dtype=np.float32)
    bo = np.asarray(bo, dtype=np.float32)
    mbo = np.asarray(mbo, dtype=np.float32)
    bf = np.asarray(bf, dtype=np.float32)

    if "nc" not in _cache:
        _cache["nc"] = _build()
    nc = _cache["nc"]

    bf16 = ml_dtypes.bfloat16
    wof = (Wo @ Wf).astype(bf16)    # [512, 256]
    mwof = (mWo @ Wf).astype(bf16)
    bias_x = bo @ Wf + bf
    bias_m = mbo @ Wf + bf

    xTs = [np.ascontiguousarray(x[b].T).astype(bf16) for b in range(2)]
    mTs = [np.ascontiguousarray(m[b].T).astype(bf16) for b in range(2)]

    in_maps = []
    for c in range(8):
        b, hp = c // 4, c % 4
        csl = slice(hp * 128, (hp + 1) * 128)
        in_maps.append({
            "xT": xTs[b], "mT": mTs[b],
            "wqk": np.ascontiguousarray(Wqk[:, csl]).astype(bf16),
            "mwqk": np.ascontiguousarray(mWqk[:, csl]).astype(bf16),
            "wv": np.ascontiguousarray(Wv[:, csl]).astype(bf16),
            "mwv": np.ascontiguousarray(mWv[:, csl]).astype(bf16),
            "wof": np.ascontiguousarray(wof[csl, :]),
            "mwof": np.ascontiguousarray(mwof[csl, :]),
        })

    res = run_bass_kernel_spmd(nc, in_maps, list(range(8)))

    out = np.empty((2, 2 * N, DM), dtype=np.float32)
    for b in range(2):
        cores = range(b * 4, b * 4 + 4)
        out[b, :N] = sum(res.results[c]["px"] for c in cores) + bias_x
        out[b, N:] = sum(res.results[c]["pm"] for c in cores) + bias_m
    return out


# revision 67
# speedup vs baseline: 2.3129x; 1.0858x over previous
"""Bidirectional cross-attention kernel for 8 Trainium2 NeuronCores.

Sharding: 16 (batch, head) units across 8 cores -> core c handles
batch b = c//4 and heads (2*(c%4), 2*(c%4)+1).

Per head on each core (all matmul data bf16):
    E   = exp(scale * qk @ m_qk^T)      [i, j] natural, exp on Act engine
          (exp's accum_out gives rowsum[i] for free)
    m-side: m_out[j, d] = sum_i E[i,j] v1[i,d]  via E tiles as stationary
            lhsT (out free = 64 -> half streaming cost); 65th ones column of
            v1 handled by separate N=1 colsum matmuls.
    x-side: needs E^T [j, i]: NA/16 of j-columns round-trip through DRAM and
            the DMA xbar transpose (14 ns per 16x128 tile in the cost model);
            the rest are recomputed as exp(sim^T) on chip.
            out[i, d] = sum_j E^T[j,i]^T mv[j,d] with E^T slabs stationary.
    Normalization = per-partition scalar multiply (DVE tensor_scalar) by
    1/rowsum / 1/colsum during the PSUM->SBUF drain, then PE transposes to
    get out^T for the folded output projection Wof = Wo @ Wf (host-folded).
Host sums the 4 per-batch partials and adds the folded biases.
"""

import numpy as np
import ml_dtypes

import concourse.bass as bass
import concourse.mybir as mybir
import concourse.tile as tile
from concourse import bacc
from concourse.bass_utils import run_bass_kernel_spmd
from concourse.masks import make_identity

F32 = mybir.dt.float32
BF16 = mybir.dt.bfloat16
EXP = mybir.ActivationFunctionType.Exp

import os

N = 2048          # sequence length (i and j)
DM = 256          # model dim
DH = 64           # head dim
NT = N // 128     # 16 row tiles
SCALE = DH ** -0.5
NA = int(os.environ.get("BXA_NA", "11"))   # E^T j-slabs via DRAM xbar
NG = NT - NA      # j-slabs of E^T recomputed on-chip (sim^T + exp)

_cache = {}


def _build():
    nc = bacc.Bacc("TRN2", target_bir_lowering=False, debug=False, num_devices=8)

    xT = nc.dram_tensor("xT", [DM, N], BF16, kind="ExternalInput")
    mT = nc.dram_tensor("mT", [DM, N], BF16, kind="ExternalInput")
    wqk = nc.dram_tensor("wqk", [DM, 128], BF16, kind="ExternalInput")
    mwqk = nc.dram_tensor("mwqk", [DM, 128], BF16, kind="ExternalInput")
    wv = nc.dram_tensor("wv", [DM, 128], BF16, kind="ExternalInput")
    mwv = nc.dram_tensor("mwv", [DM, 128], BF16, kind="ExternalInput")
    wof = nc.dram_tensor("wof", [128, DM], BF16, kind="ExternalInput")
    mwof = nc.dram_tensor("mwof", [128, DM], BF16, kind="ExternalInput")
    px = nc.dram_tensor("px", [N, DM], F32, kind="ExternalOutput")
    pm = nc.dram_tensor("pm", [N, DM], F32, kind="ExternalOutput")

    with tile.TileContext(nc) as tc:
        _emit(tc, xT, mT, wqk, mwqk, wv, mwv, wof, mwof, px, pm)
    nc.compile()
    return nc


def _emit(tc, xT, mT, wqk, mwqk, wv, mwv, wof, mwof, px, pm):
    nc = tc.nc
    import contextlib
    ctx = contextlib.ExitStack()
    with ctx:
        singles = ctx.enter_context(tc.tile_pool(name="singles", bufs=1))
        e_p = ctx.enter_context(tc.tile_pool(name="e", bufs=16))
        et_a = ctx.enter_context(tc.tile_pool(name="eta", bufs=max(NA, 1)))
        et_g = ctx.enter_context(tc.tile_pool(name="etg", bufs=max(NG, 1)))
        nrm_p = ctx.enter_context(tc.tile_pool(name="nrm", bufs=2))
        psim_p = ctx.enter_context(tc.tile_pool(name="psim", bufs=2, space="PSUM"))
        pacc_p = ctx.enter_context(tc.tile_pool(name="pacc", bufs=1, space="PSUM"))

        ident = singles.tile([128, 128], F32)
        make_identity(nc, ident)

        # ---- weights / inputs (qk weights + mT + xT first: they gate A0) ----
        wqk_sb = singles.tile([128, 2, 128], BF16)
        mwqk_sb = singles.tile([128, 2, 128], BF16)
        wv_sb = singles.tile([128, 2, 128], BF16)
        mwv_sb = singles.tile([128, 2, 128], BF16)
        for t, d in ((wqk_sb, wqk), (mwqk_sb, mwqk)):
            nc.sync.dma_start(out=t[:], in_=d.rearrange("(k p) n -> p k n", p=128))
        xT_sb = singles.tile([128, 2, N], BF16)
        mT_sb = singles.tile([128, 2, N], BF16)
        for src, dst, q in ((mT, mT_sb, 0), (mT, mT_sb, 1), (mT, mT_sb, 2),
                            (mT, mT_sb, 3), (xT, xT_sb, 0), (xT, xT_sb, 1),
                            (xT, xT_sb, 2), (xT, xT_sb, 3)):
            nc.sync.dma_start(
                out=dst[:, :, q * 512:(q + 1) * 512],
                in_=src.rearrange("(k p) n -> p k n", p=128)[:, :, q * 512:(q + 1) * 512])
        for t, d in ((wv_sb, wv), (mwv_sb, mwv)):
            nc.sync.dma_start(out=t[:], in_=d.rearrange("(k p) n -> p k n", p=128))
        wof_sb = singles.tile([64, 2, DM], BF16)
        mwof_sb = singles.tile([64, 2, DM], BF16)
        nc.sync.dma_start(out=wof_sb[:], in_=wof.rearrange("(h d) n -> d h n", d=64))
        nc.sync.dma_start(out=mwof_sb[:], in_=mwof.rearrange("(h d) n -> d h n", d=64))

        qkT = singles.tile([128, N], BF16)
        m_qkT = singles.tile([128, N], BF16)
        v_sb = singles.tile([128, NT, 2, 65], BF16)
        mv_sb = singles.tile([128, NT, 2, 65], BF16)
        rrecT = singles.tile([128, 2, NT], F32)       # 1/rowsum, i on partitions
        crecT = singles.tile([128, 2, NT], F32)       # 1/colsum, j on partitions
        outT_b = singles.tile([64, 2, N], BF16)       # x-side out^T
        m_outT_b = singles.tile([64, 2, N], BF16)     # m-side out^T
        po_m = singles.tile([128, 8, DM], F32)        # pm staging
        po_x = singles.tile([128, 8, DM], F32)        # px staging

        m_acc = pacc_p.tile([128, 1024], F32, tag="macc")   # banks 4-5
        x_acc = pacc_p.tile([128, 1024], F32, tag="xacc")   # banks 6-7
        cs = x_acc[:, 0:16]                                 # colsum sliver

        ESPLIT = int(os.environ.get("BXA_ESPLIT", "1"))
        Edram = [[nc.dram_tensor(f"edram{h}_{hf}", [N // 2, NA * 128], BF16).ap()
                  for hf in range(2)] for h in range(2)] if NA else [None, None]

        # PE p-state warmup: dummy transposes while the input DMAs stream in,
        # so the projections and first sims run at full clock
        for _ in range(14):
            nc.tensor.transpose(x_acc[:, 768:896], ident, ident)

        # ---- projections (psum borrowed from m_acc / x_acc halves) ----
        for wi, (w_sb, src, dst) in enumerate(
                ((mwqk_sb, mT_sb, m_qkT), (wqk_sb, xT_sb, qkT))):
            for t in range(4):
                pq = (m_acc if wi == 0 else x_acc)[:, (t % 2) * 512:
                                                   (t % 2) * 512 + 512]
                for kc in range(2):
                    nc.tensor.matmul(pq, w_sb[:, kc, :],
                                     src[:, kc, t * 512:(t + 1) * 512],
                                     start=(kc == 0), stop=(kc == 1))
                nc.vector.tensor_copy(dst[:, t * 512:(t + 1) * 512], pq)
        for wi, (w_sb, src, dst) in enumerate(
                ((wv_sb, xT_sb, v_sb), (mwv_sb, mT_sb, mv_sb))):
            for g in range(4):
                k4 = (wi * 4 + g) % 4
                pv = (m_acc if k4 < 2 else x_acc)[:, (k4 % 2) * 512:
                                                  (k4 % 2) * 512 + 512]
                for dt_ in range(4):
                    t = g * 4 + dt_
                    for kc in range(2):
                        nc.tensor.matmul(pv[:, dt_ * 128:(dt_ + 1) * 128],
                                         src[:, kc, t * 128:(t + 1) * 128],
                                         w_sb[:, kc, :],
                                         start=(kc == 0), stop=(kc == 1))
                nc.vector.tensor_copy(
                    dst[:, g * 4:(g + 1) * 4, :, 0:64],
                    pv.rearrange("p (t h d) -> p t h d", t=4, h=2))
            nc.vector.memset(dst[:, :, :, 64:65], 1.0)

        Et = [[None] * NT, [None] * NT]
        ET = [[None] * NT, [None] * NT]

        # PSUM start=True zeroes the WHOLE 2KB bank (zero region), so a bank
        # shared by interleaved accumulation chains must see exactly ONE
        # start (its very first matmul) and ONE stop (its very last).
        # m-side chunk: for one ic, accumulate all 16 j-tiles (+ colsum for
        # h=0, where x_acc is free during phase A)
        def m_chunk(h, ic, with_cs):
            et_t = Et[h][ic]
            for jc in range(NT):
                nc.tensor.matmul(m_acc[:, jc * 64:(jc + 1) * 64],
                                 et_t[:, jc * 128:(jc + 1) * 128],
                                 v_sb[:, ic, h, 0:64],
                                 start=(ic == 0 and jc % 8 == 0),
                                 stop=(ic == NT - 1 and jc % 8 == 7),
                                 skip_group_check=True)
                if with_cs:
                    nc.tensor.matmul(cs[:, jc:jc + 1],
                                     et_t[:, jc * 128:(jc + 1) * 128],
                                     v_sb[:, ic, h, 64:65],
                                     start=(ic == 0 and jc == 0),
                                     stop=(ic == NT - 1 and jc == NT - 1),
                                     skip_group_check=True)

        # x-side chunk: one j-slab of E^T against all 16 i-tiles
        def x_chunk(h, jt, first, last):
            et_t = ET[h][jt]
            for ic in range(NT):
                nc.tensor.matmul(x_acc[:, ic * 64:(ic + 1) * 64],
                                 et_t[:, ic * 128:(ic + 1) * 128],
                                 mv_sb[:, jt, h, 0:64],
                                 start=(first and ic % 8 == 0),
                                 stop=(last and ic % 8 == 7),
                                 skip_group_check=True)

        def cs_burst(h):
            for ic in range(NT):
                et_t = Et[h][ic]
                for jc in range(NT):
                    nc.tensor.matmul(cs[:, jc:jc + 1],
                                     et_t[:, jc * 128:(jc + 1) * 128],
                                     v_sb[:, ic, h, 64:65],
                                     start=(ic == 0 and jc == 0),
                                     stop=(ic == NT - 1 and jc == NT - 1),
                                     skip_group_check=True)

        # rowsum via N=1 matmuls over the E^T slabs, into a borrowed psim tile
        def rs_burst(h):
            rs_t = psim_p.tile([128, 1024], F32, tag="psim", name=f"rs{h}")
            rs = rs_t[:, 0:16]
            for jt in range(NT):
                et_t = ET[h][jt]
                for ic in range(NT):
                    nc.tensor.matmul(rs[:, ic:ic + 1],
                                     et_t[:, ic * 128:(ic + 1) * 128],
                                     mv_sb[:, jt, h, 64:65],
                                     start=(jt == 0 and ic == 0),
                                     stop=(jt == NT - 1 and ic == NT - 1),
                                     skip_group_check=True)
            nc.vector.reciprocal(rrecT[:, h, :], rs)

        def phase_A(h, fillers):
            fi = 0
            for ic in range(NT):
                et_t = e_p.tile([128, N], BF16, tag="et", name=f"et{h}_{ic}")
                Et[h][ic] = et_t
                for half in range(2):
                    ps = psim_p.tile([128, 1024], F32, tag="psim", name="ps")
                    for q in range(2):
                        jn = half * 2 + q
                        nc.tensor.matmul(ps[:, q * 512:(q + 1) * 512],
                                         qkT[h * 64:(h + 1) * 64,
                                             ic * 128:(ic + 1) * 128],
                                         m_qkT[h * 64:(h + 1) * 64,
                                               jn * 512:(jn + 1) * 512],
                                         start=True, stop=True)
                    nc.scalar.activation(et_t[:, half * 1024:(half + 1) * 1024],
                                         ps[:], EXP, scale=SCALE)
                if NA:
                    nc.gpsimd.dma_start(
                        out=Edram[h][ic // 8][(ic % 8) * 128:(ic % 8 + 1) * 128, :],
                        in_=et_t[:, 0:NA * 128])
                while fi <= ic and fi < len(fillers):
                    fillers[fi]()
                    fi += 1
            while fi < len(fillers):
                fillers[fi]()
                fi += 1

        def alpha_grab(h):
            for jt in range(NA):
                et_t = et_a.tile([128, N], BF16, tag="eta", name=f"eta{h}_{jt}")
                ET[h][jt] = et_t

        def alpha_loads(h, ihalf):
            if not ESPLIT and ihalf == 1:
                return
            for jt in range(NA):
                if ESPLIT:
                    # split by i: first half gated only on stores ic<8
                    nc.sync.dma_start_transpose(
                        out=ET[h][jt][:, ihalf * 1024:(ihalf + 1) * 1024],
                        in_=Edram[h][ihalf][:, jt * 128:(jt + 1) * 128])
                else:
                    for hf in range(2):
                        nc.sync.dma_start_transpose(
                            out=ET[h][jt][:, hf * 1024:(hf + 1) * 1024],
                            in_=Edram[h][hf][:, jt * 128:(jt + 1) * 128])

        def gamma_slabs(h):
            for g in range(NG):
                jt = NA + g
                et_t = et_g.tile([128, N], BF16, tag="etg", name=f"etg{h}_{jt}")
                ET[h][jt] = et_t
                for half in range(2):
                    ps = psim_p.tile([128, 1024], F32, tag="psim", name="ps")
                    for q in range(2):
                        ich = half * 2 + q
                        nc.tensor.matmul(ps[:, q * 512:(q + 1) * 512],
                                         m_qkT[h * 64:(h + 1) * 64,
                                               jt * 128:(jt + 1) * 128],
                                         qkT[h * 64:(h + 1) * 64,
                                             ich * 512:(ich + 1) * 512],
                                         start=True, stop=True)
                    nc.scalar.activation(et_t[:, half * 1024:(half + 1) * 1024],
                                         ps[:], EXP, scale=SCALE)

        def colsum_rec(h):
            nc.vector.reciprocal(crecT[:, h, :], cs[:])

        # drain + transpose one side; acc is the psum accumulator being
        # drained, tp_base the (other) psum tensor lending [64, 512] regions
        def drain_transpose(h, acc, rec, dstT, tp_base, tag, act_share=False):
            xn = nrm_p.tile([128, 1024], F32, tag="xn", name=f"xn_{tag}{h}")
            for t in range(NT):
                if act_share and t % 2 == 1:
                    nc.scalar.mul(xn[:, t * 64:(t + 1) * 64],
                                  acc[:, t * 64:(t + 1) * 64],
                                  rec[:, h, t:t + 1])
                else:
                    nc.vector.tensor_scalar_mul(xn[:, t * 64:(t + 1) * 64],
                                                acc[:, t * 64:(t + 1) * 64],
                                                rec[:, h, t:t + 1])
            for b4 in range(4):
                tp = tp_base[0:64, (b4 % 2) * 512:(b4 % 2) * 512 + 512]
                for k in range(4):
                    t = b4 * 4 + k
                    nc.tensor.transpose(tp[:, k * 128:(k + 1) * 128],
                                        xn[:, t * 64:(t + 1) * 64], ident)
                if act_share and b4 % 2 == 1:
                    nc.scalar.mul(dstT[:, h, b4 * 512:(b4 + 1) * 512], tp, 1.0)
                else:
                    nc.vector.tensor_copy(dstT[:, h, b4 * 512:(b4 + 1) * 512], tp)

        def outproj(src, w_sb, dst_dram, pregs, dma_eng, po, act_share=True):
            # copies alternate DVE/Act to halve the serial chain; DMAs go in
            # groups of 4 tiles to amortize HWDGE issue cost
            dst_r = dst_dram.rearrange("(g t p) c -> p g t c", g=4, t=4, p=128)
            nr = len(pregs)
            for t in range(NT):
                reg = pregs[t % nr]
                nc.tensor.matmul(reg, src[:, 0, t * 128:(t + 1) * 128],
                                 w_sb[:, 0, :], start=True, stop=False)
                nc.tensor.matmul(reg, src[:, 1, t * 128:(t + 1) * 128],
                                 w_sb[:, 1, :], start=False, stop=True)
                if act_share and t % 2 == 1:
                    nc.scalar.mul(po[:, t % 8, :], reg, 1.0)
                else:
                    nc.vector.tensor_copy(po[:, t % 8, :], reg)
                if t % 4 == 3:
                    g = t // 4
                    nc_dma = dma_eng
                    nc_dma.dma_start(
                        out=dst_r[:, g, :, :],
                        in_=po[:, (g % 2) * 4:(g % 2) * 4 + 4, :])

        # ================= schedule =================
        # head 0 phase A; m-side + colsum interleaved (x_acc free), with a
        # 2-iteration stagger so chunks never wait on the exp just issued
        fillers0 = [lambda: None, lambda: None] + [
            (lambda ic=ic: m_chunk(0, ic, True)) for ic in range(NT)
        ]
        phase_A(0, fillers0)
        if NA:
            alpha_grab(0)
            alpha_loads(0, 0)   # first i-half gated only on stores ic<8
            alpha_loads(0, 1)
        colsum_rec(0)
        gamma_slabs(0)
        # m-side h0 drain + transposes (borrow x_acc halves for transposes)
        drain_transpose(0, m_acc, crecT, m_outT_b, x_acc, "m")

        # head 1 phase A; fillers: X0 gamma chunks + staggered m1 chunks
        # X0 runs entirely inside A1 as fillers: gamma slabs are resident and
        # the i-split alpha loads of head 0 land early in A1
        xorder = list(range(NA, NT)) + list(range(NA))
        fillers1 = []
        for k in range(NT + 2):
            def f(k=k):
                if k < NT:
                    x_chunk(0, xorder[k], first=(k == 0),
                            last=(k == NT - 1))
                if k >= 2:
                    m_chunk(1, k - 2, False)
            fillers1.append(f)
        phase_A(1, fillers1)
        if NA:
            alpha_grab(1)
            alpha_loads(1, 0)
            alpha_loads(1, 1)
        # rowsum burst gates the h1 alpha loads (pool-slot WAR) and must also
        # precede gamma_slabs(1), whose exps reuse the gamma slabs' pool slots
        rs_burst(0)
        gamma_slabs(1)

        # x-side h0 drain (x_acc frees up), then colsum burst h1 into cs
        xn0 = nrm_p.tile([128, 1024], F32, tag="xn", name="xn_x0")
        for t in range(NT):
            nc.vector.tensor_scalar_mul(xn0[:, t * 64:(t + 1) * 64],
                                        x_acc[:, t * 64:(t + 1) * 64],
                                        rrecT[:, 0, t:t + 1])
        cs_burst(1)
        colsum_rec(1)
        # x0 transposes into x_acc halves (free after cs read)
        for b4 in range(4):
            tp = x_acc[0:64, (b4 % 2) * 512:(b4 % 2) * 512 + 512]
            for k in range(4):
                t = b4 * 4 + k
                nc.tensor.transpose(tp[:, k * 128:(k + 1) * 128],
                                    xn0[:, t * 64:(t + 1) * 64], ident)
            nc.vector.tensor_copy(outT_b[:, 0, b4 * 512:(b4 + 1) * 512], tp)
        # m-side h1 drain + transposes into x_acc halves (x0 drained, X1 not
        # yet started) so pm-outproj's m_acc quarters stay conflict-free
        drain_transpose(1, m_acc, crecT, m_outT_b, x_acc, "m")
        mq = [m_acc[:, i * 256:(i + 1) * 256] for i in range(4)]
        # x-side h1 first: it is Act-gated (gamma exps), so nothing else may
        # sit ahead of it in the in-order PE queue
        for k in range(NT):
            x_chunk(1, xorder[k], first=(k == 0), last=(k == NT - 1))
        rs_burst(1)
        # fused tail: per 4-tile group, x-side drain/transpose/project/DMA
        # round-robined with the pm projection groups
        xn1 = nrm_p.tile([128, 1024], F32, tag="xn", name="xn_x1")
        px_r = px.rearrange("(r p) c -> p r c", p=128)
        pm_r = pm.rearrange("(r p) c -> p r c", p=128)
        for b4 in range(4):
            for k in range(4):
                t = b4 * 4 + k
                if k % 2 == 1:
                    nc.scalar.mul(xn1[:, t * 64:(t + 1) * 64],
                                  x_acc[:, t * 64:(t + 1) * 64],
                                  rrecT[:, 1, t:t + 1])
                else:
                    nc.vector.tensor_scalar_mul(xn1[:, t * 64:(t + 1) * 64],
                                                x_acc[:, t * 64:(t + 1) * 64],
                                                rrecT[:, 1, t:t + 1])
            tp_t = psim_p.tile([128, 1024], F32, tag="psim", name=f"tpx{b4}")
            tp = tp_t[0:64, 0:512]
            for k in range(4):
                t = b4 * 4 + k
                nc.tensor.transpose(tp[:, k * 128:(k + 1) * 128],
                                    xn1[:, t * 64:(t + 1) * 64], ident)
            if b4 % 2 == 0:
                nc.vector.tensor_copy(outT_b[:, 1, b4 * 512:(b4 + 1) * 512], tp)
            else:
                nc.scalar.mul(outT_b[:, 1, b4 * 512:(b4 + 1) * 512], tp, 1.0)
            for k in range(4):
                t = b4 * 4 + k
                # bank B of the transpose psim tile: decoupled from pm's mq
                reg = tp_t[:, 512 + (k % 2) * 256: 768 + (k % 2) * 256]
                nc.tensor.matmul(reg, outT_b[:, 0, t * 128:(t + 1) * 128],
                                 wof_sb[:, 0, :], start=True, stop=False)
                nc.tensor.matmul(reg, outT_b[:, 1, t * 128:(t + 1) * 128],
                                 wof_sb[:, 1, :], start=False, stop=True)
                if k % 2 == 0:
                    nc.vector.tensor_copy(po_x[:, t % 8, :], reg)
                else:
                    nc.scalar.mul(po_x[:, t % 8, :], reg, 1.0)
                if k % 2 == 1:
                    nc.sync.dma_start(
                        out=px_r[:, t - 1:t + 1, :],
                        in_=po_x[:, (t - 1) % 8:(t - 1) % 8 + 2, :])
            # pm group b4 rides along
            for k in range(4):
                t = b4 * 4 + k
                reg = mq[2 * (b4 % 2) + k % 2]
                nc.tensor.matmul(reg, m_outT_b[:, 0, t * 128:(t + 1) * 128],
                                 mwof_sb[:, 0, :], start=True, stop=False)
                nc.tensor.matmul(reg, m_outT_b[:, 1, t * 128:(t + 1) * 128],
                                 mwof_sb[:, 1, :], start=False, stop=True)
                if k % 2 == 1:
                    nc.scalar.mul(po_m[:, t % 8, :], reg, 1.0)
                else:
                    nc.vector.tensor_copy(po_m[:, t % 8, :], reg)
                if k % 2 == 1:
                    nc.scalar.dma_start(
                        out=pm_r[:, t - 1:t + 1, :],
                        in_=po_m[:, (t - 1) % 8:(t - 1) % 8 + 2, :])


def kernel(x, m, Wqk, mWqk, Wv, mWv, Wo, bo, mWo, mbo, Wf, bf):
    x = np.asarray(x, dtype=np.float32)
    m = np.asarray(m, dtype=np.float32)
    Wqk = np.asarray(Wqk, dtype=np.float32)
    mWqk = np.asarray(mWqk, dtype=np.float32)
    Wv = np.asarray(Wv, dtype=np.float32)
    mWv = np.asarray(mWv, dtype=np.float32)
    Wo = np.asarray(Wo, dtype=np.float32)
    mWo = np.asarray(mWo, dtype=np.float32)
    Wf = np.asarray(Wf, dtype=np.float32)
    bo = np.asarray(bo, dtype=np.float32)
    mbo = np.asarray(mbo, dtype=np.float32)
    bf = np.asarray(bf, dtype=np.float32)

    if "nc" not in _cache:
        _cache["nc"] = _build()
    nc = _cache["nc"]

    bf16 = ml_dtypes.bfloat16
    wof = (Wo @ Wf).astype(bf16)    # [512, 256]
    mwof = (mWo @ Wf).astype(bf16)
    bias_x = bo @ Wf + bf
    bias_m = mbo @ Wf + bf

    xTs = [np.ascontiguousarray(x[b].T).astype(bf16) for b in range(2)]
    mTs = [np.ascontiguousarray(m[b].T).astype(bf16) for b in range(2)]

    in_maps = []
    for c in range(8):
        b, hp = c // 4, c % 4
        csl = slice(hp * 128, (hp + 1) * 128)
        in_maps.append({
            "xT": xTs[b], "mT": mTs[b],
            "wqk": np.ascontiguousarray(Wqk[:, csl]).astype(bf16),
            "mwqk": np.ascontiguousarray(mWqk[:, csl]).astype(bf16),
            "wv": np.ascontiguousarray(Wv[:, csl]).astype(bf16),
            "mwv": np.ascontiguousarray(mWv[:, csl]).astype(bf16),
            "wof": np.ascontiguousarray(wof[csl, :]),
            "mwof": np.ascontiguousarray(mwof[csl, :]),
        })

    res = run_bass_kernel_spmd(nc, in_maps, list(range(8)))

    out = np.empty((2, 2 * N, DM), dtype=np.float32)
    for b in range(2):
        cores = range(b * 4, b * 4 + 4)
        out[b, :N] = sum(res.results[c]["px"] for c in cores) + bias_x
        out[b, N:] = sum(res.results[c]["pm"] for c in cores) + bias_m
    return out


# revision 69
# speedup vs baseline: 2.3269x; 1.0061x over previous
"""Bidirectional cross-attention kernel for 8 Trainium2 NeuronCores.

Sharding: 16 (batch, head) units across 8 cores -> core c handles
batch b = c//4 and heads (2*(c%4), 2*(c%4)+1).

Per head on each core (all matmul data bf16):
    E   = exp(scale * qk @ m_qk^T)      [i, j] natural, exp on Act engine
          (exp's accum_out gives rowsum[i] for free)
    m-side: m_out[j, d] = sum_i E[i,j] v1[i,d]  via E tiles as stationary
            lhsT (out free = 64 -> half streaming cost); 65th ones column of
            v1 handled by separate N=1 colsum matmuls.
    x-side: needs E^T [j, i]: NA/16 of j-columns round-trip through DRAM and
            the DMA xbar transpose (14 ns per 16x128 tile in the cost model);
            the rest are recomputed as exp(sim^T) on chip.
            out[i, d] = sum_j E^T[j,i]^T mv[j,d] with E^T slabs stationary.
    Normalization = per-partition scalar multiply (DVE tensor_scalar) by
    1/rowsum / 1/colsum during the PSUM->SBUF drain, then PE transposes to
    get out^T for the folded output projection Wof = Wo @ Wf (host-folded).
Host sums the 4 per-batch partials and adds the folded biases.
"""

import numpy as np
import ml_dtypes

import concourse.bass as bass
import concourse.mybir as mybir
import concourse.tile as tile
from concourse import bacc
from concourse.bass_utils import run_bass_kernel_spmd
from concourse.masks import make_identity

F32 = mybir.dt.float32
BF16 = mybir.dt.bfloat16
EXP = mybir.ActivationFunctionType.Exp

import os

N = 2048          # sequence length (i and j)
DM = 256          # model dim
DH = 64           # head dim
NT = N // 128     # 16 row tiles
SCALE = DH ** -0.5
NA = int(os.environ.get("BXA_NA", "7"))    # E^T j-slabs via DRAM xbar
NG = NT - NA      # j-slabs of E^T recomputed on-chip (sim^T + exp)

_cache = {}


def _build():
    nc = bacc.Bacc("TRN2", target_bir_lowering=False, debug=False, num_devices=8)

    xT = nc.dram_tensor("xT", [DM, N], BF16, kind="ExternalInput")
    mT = nc.dram_tensor("mT", [DM, N], BF16, kind="ExternalInput")
    wqk = nc.dram_tensor("wqk", [DM, 128], BF16, kind="ExternalInput")
    mwqk = nc.dram_tensor("mwqk", [DM, 128], BF16, kind="ExternalInput")
    wv = nc.dram_tensor("wv", [DM, 128], BF16, kind="ExternalInput")
    mwv = nc.dram_tensor("mwv", [DM, 128], BF16, kind="ExternalInput")
    wof = nc.dram_tensor("wof", [128, DM], BF16, kind="ExternalInput")
    mwof = nc.dram_tensor("mwof", [128, DM], BF16, kind="ExternalInput")
    px = nc.dram_tensor("px", [N, DM], F32, kind="ExternalOutput")
    pm = nc.dram_tensor("pm", [N, DM], F32, kind="ExternalOutput")

    with tile.TileContext(nc) as tc:
        _emit(tc, xT, mT, wqk, mwqk, wv, mwv, wof, mwof, px, pm)
    nc.compile()
    return nc


def _emit(tc, xT, mT, wqk, mwqk, wv, mwv, wof, mwof, px, pm):
    nc = tc.nc
    import contextlib
    ctx = contextlib.ExitStack()
    with ctx:
        singles = ctx.enter_context(tc.tile_pool(name="singles", bufs=1))
        e_p = ctx.enter_context(tc.tile_pool(name="e", bufs=16))
        et_a = ctx.enter_context(tc.tile_pool(name="eta", bufs=max(NA, 1)))
        et_g = ctx.enter_context(tc.tile_pool(name="etg", bufs=max(NG, 1)))
        nrm_p = ctx.enter_context(tc.tile_pool(name="nrm", bufs=2))
        psim_p = ctx.enter_context(tc.tile_pool(name="psim", bufs=2, space="PSUM"))
        pacc_p = ctx.enter_context(tc.tile_pool(name="pacc", bufs=1, space="PSUM"))

        ident = singles.tile([128, 128], F32)
        make_identity(nc, ident)

        # ---- weights / inputs (qk weights + mT + xT first: they gate A0) ----
        wqk_sb = singles.tile([128, 2, 128], BF16)
        mwqk_sb = singles.tile([128, 2, 128], BF16)
        wv_sb = singles.tile([128, 2, 128], BF16)
        mwv_sb = singles.tile([128, 2, 128], BF16)
        for t, d in ((wqk_sb, wqk), (mwqk_sb, mwqk)):
            nc.sync.dma_start(out=t[:], in_=d.rearrange("(k p) n -> p k n", p=128))
        xT_sb = singles.tile([128, 2, N], BF16)
        mT_sb = singles.tile([128, 2, N], BF16)
        for src, dst, q in ((mT, mT_sb, 0), (mT, mT_sb, 1), (mT, mT_sb, 2),
                            (mT, mT_sb, 3), (xT, xT_sb, 0), (xT, xT_sb, 1),
                            (xT, xT_sb, 2), (xT, xT_sb, 3)):
            nc.sync.dma_start(
                out=dst[:, :, q * 512:(q + 1) * 512],
                in_=src.rearrange("(k p) n -> p k n", p=128)[:, :, q * 512:(q + 1) * 512])
        for t, d in ((wv_sb, wv), (mwv_sb, mwv)):
            nc.sync.dma_start(out=t[:], in_=d.rearrange("(k p) n -> p k n", p=128))
        wof_sb = singles.tile([64, 2, DM], BF16)
        mwof_sb = singles.tile([64, 2, DM], BF16)
        nc.sync.dma_start(out=wof_sb[:], in_=wof.rearrange("(h d) n -> d h n", d=64))
        nc.sync.dma_start(out=mwof_sb[:], in_=mwof.rearrange("(h d) n -> d h n", d=64))

        qkT = singles.tile([128, N], BF16)
        m_qkT = singles.tile([128, N], BF16)
        v_sb = singles.tile([128, NT, 2, 65], BF16)
        mv_sb = singles.tile([128, NT, 2, 65], BF16)
        rrecT = singles.tile([128, 2, NT], F32)       # 1/rowsum, i on partitions
        crecT = singles.tile([128, 2, NT], F32)       # 1/colsum, j on partitions
        outT_b = singles.tile([64, 2, N], BF16)       # x-side out^T
        m_outT_b = singles.tile([64, 2, N], BF16)     # m-side out^T
        po_m = singles.tile([128, 8, DM], F32)        # pm staging
        po_x = singles.tile([128, 8, DM], F32)        # px staging

        m_acc = pacc_p.tile([128, 1024], F32, tag="macc")   # banks 4-5
        x_acc = pacc_p.tile([128, 1024], F32, tag="xacc")   # banks 6-7
        cs = x_acc[:, 0:16]                                 # colsum sliver

        ESPLIT = int(os.environ.get("BXA_ESPLIT", "1"))
        Edram = [[nc.dram_tensor(f"edram{h}_{hf}", [N // 2, NA * 128], BF16).ap()
                  for hf in range(2)] for h in range(2)] if NA else [None, None]

        # PE p-state warmup: dummy transposes while the input DMAs stream in,
        # so the projections and first sims run at full clock
        for _ in range(14):
            nc.tensor.transpose(x_acc[:, 768:896], ident, ident)

        # ---- projections (psum borrowed from m_acc / x_acc halves) ----
        for wi, (w_sb, src, dst) in enumerate(
                ((mwqk_sb, mT_sb, m_qkT), (wqk_sb, xT_sb, qkT))):
            for t in range(4):
                pq = (m_acc if wi == 0 else x_acc)[:, (t % 2) * 512:
                                                   (t % 2) * 512 + 512]
                for kc in range(2):
                    nc.tensor.matmul(pq, w_sb[:, kc, :],
                                     src[:, kc, t * 512:(t + 1) * 512],
                                     start=(kc == 0), stop=(kc == 1))
                nc.vector.tensor_copy(dst[:, t * 512:(t + 1) * 512], pq)
        for wi, (w_sb, src, dst) in enumerate(
                ((wv_sb, xT_sb, v_sb), (mwv_sb, mT_sb, mv_sb))):
            for g in range(4):
                k4 = (wi * 4 + g) % 4
                pv = (m_acc if k4 < 2 else x_acc)[:, (k4 % 2) * 512:
                                                  (k4 % 2) * 512 + 512]
                for dt_ in range(4):
                    t = g * 4 + dt_
                    for kc in range(2):
                        nc.tensor.matmul(pv[:, dt_ * 128:(dt_ + 1) * 128],
                                         src[:, kc, t * 128:(t + 1) * 128],
                                         w_sb[:, kc, :],
                                         start=(kc == 0), stop=(kc == 1))
                nc.vector.tensor_copy(
                    dst[:, g * 4:(g + 1) * 4, :, 0:64],
                    pv.rearrange("p (t h d) -> p t h d", t=4, h=2))
            nc.vector.memset(dst[:, :, :, 64:65], 1.0)

        Et = [[None] * NT, [None] * NT]
        ET = [[None] * NT, [None] * NT]

        # PSUM start=True zeroes the WHOLE 2KB bank (zero region), so a bank
        # shared by interleaved accumulation chains must see exactly ONE
        # start (its very first matmul) and ONE stop (its very last).
        # m-side chunk: for one ic, accumulate all 16 j-tiles (+ colsum for
        # h=0, where x_acc is free during phase A)
        def m_chunk(h, ic, with_cs):
            et_t = Et[h][ic]
            for jc in range(NT):
                nc.tensor.matmul(m_acc[:, jc * 64:(jc + 1) * 64],
                                 et_t[:, jc * 128:(jc + 1) * 128],
                                 v_sb[:, ic, h, 0:64],
                                 start=(ic == 0 and jc % 8 == 0),
                                 stop=(ic == NT - 1 and jc % 8 == 7),
                                 skip_group_check=True)
                if with_cs:
                    nc.tensor.matmul(cs[:, jc:jc + 1],
                                     et_t[:, jc * 128:(jc + 1) * 128],
                                     v_sb[:, ic, h, 64:65],
                                     start=(ic == 0 and jc == 0),
                                     stop=(ic == NT - 1 and jc == NT - 1),
                                     skip_group_check=True)

        # x-side chunk: one j-slab of E^T against all 16 i-tiles
        def x_chunk(h, jt, first, last):
            et_t = ET[h][jt]
            for ic in range(NT):
                nc.tensor.matmul(x_acc[:, ic * 64:(ic + 1) * 64],
                                 et_t[:, ic * 128:(ic + 1) * 128],
                                 mv_sb[:, jt, h, 0:64],
                                 start=(first and ic % 8 == 0),
                                 stop=(last and ic % 8 == 7),
                                 skip_group_check=True)

        def cs_burst(h):
            for ic in range(NT):
                et_t = Et[h][ic]
                for jc in range(NT):
                    nc.tensor.matmul(cs[:, jc:jc + 1],
                                     et_t[:, jc * 128:(jc + 1) * 128],
                                     v_sb[:, ic, h, 64:65],
                                     start=(ic == 0 and jc == 0),
                                     stop=(ic == NT - 1 and jc == NT - 1),
                                     skip_group_check=True)

        # rowsum via N=1 matmuls over the E^T slabs, into a borrowed psim tile
        def rs_burst(h):
            rs_t = psim_p.tile([128, 1024], F32, tag="psim", name=f"rs{h}")
            rs = rs_t[:, 0:16]
            for jt in range(NT):
                et_t = ET[h][jt]
                for ic in range(NT):
                    nc.tensor.matmul(rs[:, ic:ic + 1],
                                     et_t[:, ic * 128:(ic + 1) * 128],
                                     mv_sb[:, jt, h, 64:65],
                                     start=(jt == 0 and ic == 0),
                                     stop=(jt == NT - 1 and ic == NT - 1),
                                     skip_group_check=True)
            nc.vector.reciprocal(rrecT[:, h, :], rs)

        def phase_A(h, fillers):
            fi = 0
            for ic in range(NT):
                et_t = e_p.tile([128, N], BF16, tag="et", name=f"et{h}_{ic}")
                Et[h][ic] = et_t
                for half in range(2):
                    ps = psim_p.tile([128, 1024], F32, tag="psim", name="ps")
                    for q in range(2):
                        jn = half * 2 + q
                        nc.tensor.matmul(ps[:, q * 512:(q + 1) * 512],
                                         qkT[h * 64:(h + 1) * 64,
                                             ic * 128:(ic + 1) * 128],
                                         m_qkT[h * 64:(h + 1) * 64,
                                               jn * 512:(jn + 1) * 512],
                                         start=True, stop=True)
                    nc.scalar.activation(et_t[:, half * 1024:(half + 1) * 1024],
                                         ps[:], EXP, scale=SCALE)
                if NA:
                    nc.gpsimd.dma_start(
                        out=Edram[h][ic // 8][(ic % 8) * 128:(ic % 8 + 1) * 128, :],
                        in_=et_t[:, 0:NA * 128])
                while fi <= ic and fi < len(fillers):
                    fillers[fi]()
                    fi += 1
            while fi < len(fillers):
                fillers[fi]()
                fi += 1

        def alpha_grab(h):
            for jt in range(NA):
                et_t = et_a.tile([128, N], BF16, tag="eta", name=f"eta{h}_{jt}")
                ET[h][jt] = et_t

        def alpha_loads(h, ihalf):
            if not ESPLIT and ihalf == 1:
                return
            for jt in range(NA):
                if ESPLIT:
                    # split by i: first half gated only on stores ic<8
                    nc.sync.dma_start_transpose(
                        out=ET[h][jt][:, ihalf * 1024:(ihalf + 1) * 1024],
                        in_=Edram[h][ihalf][:, jt * 128:(jt + 1) * 128])
                else:
                    for hf in range(2):
                        nc.sync.dma_start_transpose(
                            out=ET[h][jt][:, hf * 1024:(hf + 1) * 1024],
                            in_=Edram[h][hf][:, jt * 128:(jt + 1) * 128])

        def gamma_slabs(h):
            for g in range(NG):
                jt = NA + g
                et_t = et_g.tile([128, N], BF16, tag="etg", name=f"etg{h}_{jt}")
                ET[h][jt] = et_t
                for half in range(2):
                    ps = psim_p.tile([128, 1024], F32, tag="psim", name="ps")
                    for q in range(2):
                        ich = half * 2 + q
                        nc.tensor.matmul(ps[:, q * 512:(q + 1) * 512],
                                         m_qkT[h * 64:(h + 1) * 64,
                                               jt * 128:(jt + 1) * 128],
                                         qkT[h * 64:(h + 1) * 64,
                                             ich * 512:(ich + 1) * 512],
                                         start=True, stop=True)
                    nc.scalar.activation(et_t[:, half * 1024:(half + 1) * 1024],
                                         ps[:], EXP, scale=SCALE)

        def colsum_rec(h):
            nc.vector.reciprocal(crecT[:, h, :], cs[:])

        # drain + transpose one side; acc is the psum accumulator being
        # drained, tp_base the (other) psum tensor lending [64, 512] regions
        def drain_transpose(h, acc, rec, dstT, tp_base, tag, act_share=False):
            xn = nrm_p.tile([128, 1024], F32, tag="xn", name=f"xn_{tag}{h}")
            for t in range(NT):
                if act_share and t % 2 == 1:
                    nc.scalar.mul(xn[:, t * 64:(t + 1) * 64],
                                  acc[:, t * 64:(t + 1) * 64],
                                  rec[:, h, t:t + 1])
                else:
                    nc.vector.tensor_scalar_mul(xn[:, t * 64:(t + 1) * 64],
                                                acc[:, t * 64:(t + 1) * 64],
                                                rec[:, h, t:t + 1])
            for b4 in range(4):
                tp = tp_base[0:64, (b4 % 2) * 512:(b4 % 2) * 512 + 512]
                for k in range(4):
                    t = b4 * 4 + k
                    nc.tensor.transpose(tp[:, k * 128:(k + 1) * 128],
                                        xn[:, t * 64:(t + 1) * 64], ident)
                if act_share and b4 % 2 == 1:
                    nc.scalar.mul(dstT[:, h, b4 * 512:(b4 + 1) * 512], tp, 1.0)
                else:
                    nc.vector.tensor_copy(dstT[:, h, b4 * 512:(b4 + 1) * 512], tp)

        def outproj(src, w_sb, dst_dram, pregs, dma_eng, po, act_share=True):
            # copies alternate DVE/Act to halve the serial chain; DMAs go in
            # groups of 4 tiles to amortize HWDGE issue cost
            dst_r = dst_dram.rearrange("(g t p) c -> p g t c", g=4, t=4, p=128)
            nr = len(pregs)
            for t in range(NT):
                reg = pregs[t % nr]
                nc.tensor.matmul(reg, src[:, 0, t * 128:(t + 1) * 128],
                                 w_sb[:, 0, :], start=True, stop=False)
                nc.tensor.matmul(reg, src[:, 1, t * 128:(t + 1) * 128],
                                 w_sb[:, 1, :], start=False, stop=True)
                if act_share and t % 2 == 1:
                    nc.scalar.mul(po[:, t % 8, :], reg, 1.0)
                else:
                    nc.vector.tensor_copy(po[:, t % 8, :], reg)
                if t % 4 == 3:
                    g = t // 4
                    nc_dma = dma_eng
                    nc_dma.dma_start(
                        out=dst_r[:, g, :, :],
                        in_=po[:, (g % 2) * 4:(g % 2) * 4 + 4, :])

        # ================= schedule =================
        # head 0 phase A; m-side + colsum interleaved (x_acc free), with a
        # 2-iteration stagger so chunks never wait on the exp just issued
        fillers0 = [lambda: None, lambda: None] + [
            (lambda ic=ic: m_chunk(0, ic, True)) for ic in range(NT)
        ]
        phase_A(0, fillers0)
        if NA:
            alpha_grab(0)
            alpha_loads(0, 0)   # first i-half gated only on stores ic<8
            alpha_loads(0, 1)
        colsum_rec(0)
        gamma_slabs(0)
        # m-side h0 drain + transposes (borrow x_acc halves for transposes)
        drain_transpose(0, m_acc, crecT, m_outT_b, x_acc, "m")

        # head 1 phase A; fillers: X0 gamma chunks + staggered m1 chunks
        # X0 runs entirely inside A1 as fillers: gamma slabs are resident and
        # the i-split alpha loads of head 0 land early in A1
        xorder = list(range(NA, NT)) + list(range(NA))
        fillers1 = []
        for k in range(NT + 2):
            def f(k=k):
                if k < NT:
                    x_chunk(0, xorder[k], first=(k == 0),
                            last=(k == NT - 1))
                if k >= 2:
                    m_chunk(1, k - 2, False)
            fillers1.append(f)
        phase_A(1, fillers1)
        if NA:
            alpha_grab(1)
            alpha_loads(1, 0)
            alpha_loads(1, 1)
        # rowsum burst gates the h1 alpha loads (pool-slot WAR) and must also
        # precede gamma_slabs(1), whose exps reuse the gamma slabs' pool slots
        rs_burst(0)
        gamma_slabs(1)

        # x-side h0 drain (x_acc frees up), then colsum burst h1 into cs
        xn0 = nrm_p.tile([128, 1024], F32, tag="xn", name="xn_x0")
        for t in range(NT):
            nc.vector.tensor_scalar_mul(xn0[:, t * 64:(t + 1) * 64],
                                        x_acc[:, t * 64:(t + 1) * 64],
                                        rrecT[:, 0, t:t + 1])
        cs_burst(1)
        colsum_rec(1)
        # x0 transposes into x_acc halves (free after cs read)
        for b4 in range(4):
            tp = x_acc[0:64, (b4 % 2) * 512:(b4 % 2) * 512 + 512]
            for k in range(4):
                t = b4 * 4 + k
                nc.tensor.transpose(tp[:, k * 128:(k + 1) * 128],
                                    xn0[:, t * 64:(t + 1) * 64], ident)
            nc.vector.tensor_copy(outT_b[:, 0, b4 * 512:(b4 + 1) * 512], tp)
        # m-side h1 drain + transposes into x_acc halves (x0 drained, X1 not
        # yet started) so pm-outproj's m_acc quarters stay conflict-free
        drain_transpose(1, m_acc, crecT, m_outT_b, x_acc, "m")
        mq = [m_acc[:, i * 256:(i + 1) * 256] for i in range(4)]
        # x-side h1 first: it is Act-gated (gamma exps), so nothing else may
        # sit ahead of it in the in-order PE queue; rowsum matmuls ride along
        # per slab so the reciprocal is ready the moment X1 closes
        rs1_t = psim_p.tile([128, 1024], F32, tag="psim", name="rs1")
        rs1 = rs1_t[:, 0:16]
        for k in range(NT):
            jt = xorder[k]
            x_chunk(1, jt, first=(k == 0), last=(k == NT - 1))
            for ic in range(NT):
                nc.tensor.matmul(rs1[:, ic:ic + 1],
                                 ET[1][jt][:, ic * 128:(ic + 1) * 128],
                                 mv_sb[:, jt, 1, 64:65],
                                 start=(k == 0 and ic == 0),
                                 stop=(k == NT - 1 and ic == NT - 1),
                                 skip_group_check=True)
        nc.vector.reciprocal(rrecT[:, 1, :], rs1)
        # fused tail: per 4-tile group, x-side drain/transpose/project/DMA
        # round-robined with the pm projection groups
        xn1 = nrm_p.tile([128, 1024], F32, tag="xn", name="xn_x1")
        px_r = px.rearrange("(r p) c -> p r c", p=128)
        pm_r = pm.rearrange("(r p) c -> p r c", p=128)
        for b4 in range(4):
            for k in range(4):
                t = b4 * 4 + k
                if k % 2 == 1:
                    nc.scalar.mul(xn1[:, t * 64:(t + 1) * 64],
                                  x_acc[:, t * 64:(t + 1) * 64],
                                  rrecT[:, 1, t:t + 1])
                else:
                    nc.vector.tensor_scalar_mul(xn1[:, t * 64:(t + 1) * 64],
                                                x_acc[:, t * 64:(t + 1) * 64],
                                                rrecT[:, 1, t:t + 1])
            tp_t = psim_p.tile([128, 1024], F32, tag="psim", name=f"tpx{b4}")
            tp = tp_t[0:64, 0:512]
            for k in range(4):
                t = b4 * 4 + k
                nc.tensor.transpose(tp[:, k * 128:(k + 1) * 128],
                                    xn1[:, t * 64:(t + 1) * 64], ident)
            if b4 % 2 == 0:
                nc.vector.tensor_copy(outT_b[:, 1, b4 * 512:(b4 + 1) * 512], tp)
            else:
                nc.scalar.mul(outT_b[:, 1, b4 * 512:(b4 + 1) * 512], tp, 1.0)
            for k in range(4):
                t = b4 * 4 + k
                # bank B of the transpose psim tile: decoupled from pm's mq
                reg = tp_t[:, 512 + (k % 2) * 256: 768 + (k % 2) * 256]
                nc.tensor.matmul(reg, outT_b[:, 0, t * 128:(t + 1) * 128],
                                 wof_sb[:, 0, :], start=True, stop=False)
                nc.tensor.matmul(reg, outT_b[:, 1, t * 128:(t + 1) * 128],
                                 wof_sb[:, 1, :], start=False, stop=True)
                if k % 2 == 0:
                    nc.vector.tensor_copy(po_x[:, t % 8, :], reg)
                else:
                    nc.scalar.mul(po_x[:, t % 8, :], reg, 1.0)
                if k % 2 == 1:
                    nc.sync.dma_start(
                        out=px_r[:, t - 1:t + 1, :],
                        in_=po_x[:, (t - 1) % 8:(t - 1) % 8 + 2, :])
            # pm group b4 rides along
            for k in range(4):
                t = b4 * 4 + k
                reg = mq[2 * (b4 % 2) + k % 2]
                nc.tensor.matmul(reg, m_outT_b[:, 0, t * 128:(t + 1) * 128],
                                 mwof_sb[:, 0, :], start=True, stop=False)
                nc.tensor.matmul(reg, m_outT_b[:, 1, t * 128:(t + 1) * 128],
                                 mwof_sb[:, 1, :], start=False, stop=True)
                if k % 2 == 1:
                    nc.scalar.mul(po_m[:, t % 8, :], reg, 1.0)
                else:
                    nc.vector.tensor_copy(po_m[:, t % 8, :], reg)
                if k % 2 == 1:
                    nc.scalar.dma_start(
                        out=pm_r[:, t - 1:t + 1, :],
                        in_=po_m[:, (t - 1) % 8:(t - 1) % 8 + 2, :])


def kernel(x, m, Wqk, mWqk, Wv, mWv, Wo, bo, mWo, mbo, Wf, bf):
    x = np.asarray(x, dtype=np.float32)
    m = np.asarray(m, dtype=np.float32)
    Wqk = np.asarray(Wqk, dtype=np.float32)
    mWqk = np.asarray(mWqk, dtype=np.float32)
    Wv = np.asarray(Wv, dtype=np.float32)
    mWv = np.asarray(mWv, dtype=np.float32)
    Wo = np.asarray(Wo, dtype=np.float32)
    mWo = np.asarray(mWo, dtype=np.float32)
    Wf = np.asarray(Wf, dtype=np.float32)
    bo = np.asarray(bo, dtype=np.float32)
    mbo = np.asarray(mbo, dtype=np.float32)
    bf = np.asarray(bf, dtype=np.float32)

    if "nc" not in _cache:
        _cache["nc"] = _build()
    nc = _cache["nc"]

    bf16 = ml_dtypes.bfloat16
    wof = (Wo @ Wf).astype(bf16)    # [512, 256]
    mwof = (mWo @ Wf).astype(bf16)
    bias_x = bo @ Wf + bf
    bias_m = mbo @ Wf + bf

    xTs = [np.ascontiguousarray(x[b].T).astype(bf16) for b in range(2)]
    mTs = [np.ascontiguousarray(m[b].T).astype(bf16) for b in range(2)]

    in_maps = []
    for c in range(8):
        b, hp = c // 4, c % 4
        csl = slice(hp * 128, (hp + 1) * 128)
        in_maps.append({
            "xT": xTs[b], "mT": mTs[b],
            "wqk": np.ascontiguousarray(Wqk[:, csl]).astype(bf16),
            "mwqk": np.ascontiguousarray(mWqk[:, csl]).astype(bf16),
            "wv": np.ascontiguousarray(Wv[:, csl]).astype(bf16),
            "mwv": np.ascontiguousarray(mWv[:, csl]).astype(bf16),
            "wof": np.ascontiguousarray(wof[csl, :]),
            "mwof": np.ascontiguousarray(mwof[csl, :]),
        })

    res = run_bass_kernel_spmd(nc, in_maps, list(range(8)))

    out = np.empty((2, 2 * N, DM), dtype=np.float32)
    for b in range(2):
        cores = range(b * 4, b * 4 + 4)
        out[b, :N] = sum(res.results[c]["px"] for c in cores) + bias_x
        out[b, N:] = sum(res.results[c]["pm"] for c in cores) + bias_m
    return out


# revision 71
# speedup vs baseline: 2.3450x; 1.0078x over previous
"""Bidirectional cross-attention kernel for 8 Trainium2 NeuronCores.

Sharding: 16 (batch, head) units across 8 cores -> core c handles
batch b = c//4 and heads (2*(c%4), 2*(c%4)+1).

Per head on each core (all matmul data bf16):
    E   = exp(scale * qk @ m_qk^T)      [i, j] natural, exp on Act engine
          (exp's accum_out gives rowsum[i] for free)
    m-side: m_out[j, d] = sum_i E[i,j] v1[i,d]  via E tiles as stationary
            lhsT (out free = 64 -> half streaming cost); 65th ones column of
            v1 handled by separate N=1 colsum matmuls.
    x-side: needs E^T [j, i]: NA/16 of j-columns round-trip through DRAM and
            the DMA xbar transpose (14 ns per 16x128 tile in the cost model);
            the rest are recomputed as exp(sim^T) on chip.
            out[i, d] = sum_j E^T[j,i]^T mv[j,d] with E^T slabs stationary.
    Normalization = per-partition scalar multiply (DVE tensor_scalar) by
    1/rowsum / 1/colsum during the PSUM->SBUF drain, then PE transposes to
    get out^T for the folded output projection Wof = Wo @ Wf (host-folded).
Host sums the 4 per-batch partials and adds the folded biases.
"""

import numpy as np
import ml_dtypes

import concourse.bass as bass
import concourse.mybir as mybir
import concourse.tile as tile
from concourse import bacc
from concourse.bass_utils import run_bass_kernel_spmd
from concourse.masks import make_identity

F32 = mybir.dt.float32
BF16 = mybir.dt.bfloat16
EXP = mybir.ActivationFunctionType.Exp

import os

N = 2048          # sequence length (i and j)
DM = 256          # model dim
DH = 64           # head dim
NT = N // 128     # 16 row tiles
SCALE = DH ** -0.5
NA = int(os.environ.get("BXA_NA", "7"))    # E^T j-slabs via DRAM xbar
NG = NT - NA      # j-slabs of E^T recomputed on-chip (sim^T + exp)

_cache = {}


def _build():
    nc = bacc.Bacc("TRN2", target_bir_lowering=False, debug=False, num_devices=8)

    xT = nc.dram_tensor("xT", [DM, N], BF16, kind="ExternalInput")
    mT = nc.dram_tensor("mT", [DM, N], BF16, kind="ExternalInput")
    wqk = nc.dram_tensor("wqk", [DM, 128], BF16, kind="ExternalInput")
    mwqk = nc.dram_tensor("mwqk", [DM, 128], BF16, kind="ExternalInput")
    wv = nc.dram_tensor("wv", [DM, 128], BF16, kind="ExternalInput")
    mwv = nc.dram_tensor("mwv", [DM, 128], BF16, kind="ExternalInput")
    wof = nc.dram_tensor("wof", [128, DM], BF16, kind="ExternalInput")
    mwof = nc.dram_tensor("mwof", [128, DM], BF16, kind="ExternalInput")
    px = nc.dram_tensor("px", [N, DM], F32, kind="ExternalOutput")
    pm = nc.dram_tensor("pm", [N, DM], F32, kind="ExternalOutput")

    with tile.TileContext(nc) as tc:
        _emit(tc, xT, mT, wqk, mwqk, wv, mwv, wof, mwof, px, pm)
    nc.compile()
    return nc


def _emit(tc, xT, mT, wqk, mwqk, wv, mwv, wof, mwof, px, pm):
    nc = tc.nc
    import contextlib
    ctx = contextlib.ExitStack()
    with ctx:
        singles = ctx.enter_context(tc.tile_pool(name="singles", bufs=1))
        e_p = ctx.enter_context(tc.tile_pool(name="e", bufs=16))
        et_a = ctx.enter_context(tc.tile_pool(name="eta", bufs=max(NA, 1)))
        et_g = ctx.enter_context(tc.tile_pool(name="etg", bufs=max(NG, 1)))
        nrm_p = ctx.enter_context(tc.tile_pool(name="nrm", bufs=2))
        psim_p = ctx.enter_context(tc.tile_pool(name="psim", bufs=2, space="PSUM"))
        pacc_p = ctx.enter_context(tc.tile_pool(name="pacc", bufs=1, space="PSUM"))

        ident = singles.tile([128, 128], F32)
        make_identity(nc, ident)

        # ---- weights / inputs (qk weights + mT + xT first: they gate A0) ----
        wqk_sb = singles.tile([128, 2, 128], BF16)
        mwqk_sb = singles.tile([128, 2, 128], BF16)
        wv_sb = singles.tile([128, 2, 128], BF16)
        mwv_sb = singles.tile([128, 2, 128], BF16)
        for t, d in ((wqk_sb, wqk), (mwqk_sb, mwqk)):
            nc.sync.dma_start(out=t[:], in_=d.rearrange("(k p) n -> p k n", p=128))
        xT_sb = singles.tile([128, 2, N], BF16)
        mT_sb = singles.tile([128, 2, N], BF16)
        for src, dst, q in ((mT, mT_sb, 0), (xT, xT_sb, 0), (mT, mT_sb, 1),
                            (mT, mT_sb, 2), (mT, mT_sb, 3), (xT, xT_sb, 1),
                            (xT, xT_sb, 2), (xT, xT_sb, 3)):
            nc.sync.dma_start(
                out=dst[:, :, q * 512:(q + 1) * 512],
                in_=src.rearrange("(k p) n -> p k n", p=128)[:, :, q * 512:(q + 1) * 512])
        for t, d in ((wv_sb, wv), (mwv_sb, mwv)):
            nc.sync.dma_start(out=t[:], in_=d.rearrange("(k p) n -> p k n", p=128))
        wof_sb = singles.tile([64, 2, DM], BF16)
        mwof_sb = singles.tile([64, 2, DM], BF16)
        nc.sync.dma_start(out=wof_sb[:], in_=wof.rearrange("(h d) n -> d h n", d=64))
        nc.sync.dma_start(out=mwof_sb[:], in_=mwof.rearrange("(h d) n -> d h n", d=64))

        qkT = singles.tile([128, N], BF16)
        m_qkT = singles.tile([128, N], BF16)
        v_sb = singles.tile([128, NT, 2, 65], BF16)
        mv_sb = singles.tile([128, NT, 2, 65], BF16)
        rrecT = singles.tile([128, 2, NT], F32)       # 1/rowsum, i on partitions
        crecT = singles.tile([128, 2, NT], F32)       # 1/colsum, j on partitions
        outT_b = singles.tile([64, 2, N], BF16)       # x-side out^T
        m_outT_b = singles.tile([64, 2, N], BF16)     # m-side out^T
        po_m = singles.tile([128, 8, DM], F32)        # pm staging
        po_x = singles.tile([128, 8, DM], F32)        # px staging

        m_acc = pacc_p.tile([128, 1024], F32, tag="macc")   # banks 4-5
        x_acc = pacc_p.tile([128, 1024], F32, tag="xacc")   # banks 6-7
        cs = x_acc[:, 0:16]                                 # colsum sliver

        ESPLIT = int(os.environ.get("BXA_ESPLIT", "1"))
        Edram = [[nc.dram_tensor(f"edram{h}_{hf}", [N // 2, NA * 128], BF16).ap()
                  for hf in range(2)] for h in range(2)] if NA else [None, None]

        # PE p-state warmup: dummy transposes while the input DMAs stream in,
        # so the projections and first sims run at full clock
        for _ in range(14):
            nc.tensor.transpose(x_acc[:, 768:896], ident, ident)

        # ---- projections (psum borrowed from m_acc / x_acc halves) ----
        qk_jobs = [(mwqk_sb, mT_sb, m_qkT, m_acc, 0), (wqk_sb, xT_sb, qkT, x_acc, 0),
                   (mwqk_sb, mT_sb, m_qkT, m_acc, 1), (mwqk_sb, mT_sb, m_qkT, m_acc, 2),
                   (mwqk_sb, mT_sb, m_qkT, m_acc, 3), (wqk_sb, xT_sb, qkT, x_acc, 1),
                   (wqk_sb, xT_sb, qkT, x_acc, 2), (wqk_sb, xT_sb, qkT, x_acc, 3)]
        for w_sb, src, dst, pb, t in qk_jobs:
            pq = pb[:, (t % 2) * 512:(t % 2) * 512 + 512]
            for kc in range(2):
                nc.tensor.matmul(pq, w_sb[:, kc, :],
                                 src[:, kc, t * 512:(t + 1) * 512],
                                 start=(kc == 0), stop=(kc == 1))
            nc.vector.tensor_copy(dst[:, t * 512:(t + 1) * 512], pq)
        for wi, (w_sb, src, dst) in enumerate(
                ((wv_sb, xT_sb, v_sb), (mwv_sb, mT_sb, mv_sb))):
            for g in range(4):
                k4 = (wi * 4 + g) % 4
                pv = (m_acc if k4 < 2 else x_acc)[:, (k4 % 2) * 512:
                                                  (k4 % 2) * 512 + 512]
                for dt_ in range(4):
                    t = g * 4 + dt_
                    for kc in range(2):
                        nc.tensor.matmul(pv[:, dt_ * 128:(dt_ + 1) * 128],
                                         src[:, kc, t * 128:(t + 1) * 128],
                                         w_sb[:, kc, :],
                                         start=(kc == 0), stop=(kc == 1))
                nc.vector.tensor_copy(
                    dst[:, g * 4:(g + 1) * 4, :, 0:64],
                    pv.rearrange("p (t h d) -> p t h d", t=4, h=2))
            nc.vector.memset(dst[:, :, :, 64:65], 1.0)

        Et = [[None] * NT, [None] * NT]
        ET = [[None] * NT, [None] * NT]

        # PSUM start=True zeroes the WHOLE 2KB bank (zero region), so a bank
        # shared by interleaved accumulation chains must see exactly ONE
        # start (its very first matmul) and ONE stop (its very last).
        # m-side chunk: for one ic, accumulate all 16 j-tiles (+ colsum for
        # h=0, where x_acc is free during phase A)
        def m_chunk(h, ic, with_cs):
            et_t = Et[h][ic]
            for jc in range(NT):
                nc.tensor.matmul(m_acc[:, jc * 64:(jc + 1) * 64],
                                 et_t[:, jc * 128:(jc + 1) * 128],
                                 v_sb[:, ic, h, 0:64],
                                 start=(ic == 0 and jc % 8 == 0),
                                 stop=(ic == NT - 1 and jc % 8 == 7),
                                 skip_group_check=True)
                if with_cs:
                    nc.tensor.matmul(cs[:, jc:jc + 1],
                                     et_t[:, jc * 128:(jc + 1) * 128],
                                     v_sb[:, ic, h, 64:65],
                                     start=(ic == 0 and jc == 0),
                                     stop=(ic == NT - 1 and jc == NT - 1),
                                     skip_group_check=True)

        # x-side chunk: one j-slab of E^T against all 16 i-tiles
        def x_chunk(h, jt, first, last):
            et_t = ET[h][jt]
            for ic in range(NT):
                nc.tensor.matmul(x_acc[:, ic * 64:(ic + 1) * 64],
                                 et_t[:, ic * 128:(ic + 1) * 128],
                                 mv_sb[:, jt, h, 0:64],
                                 start=(first and ic % 8 == 0),
                                 stop=(last and ic % 8 == 7),
                                 skip_group_check=True)

        def cs_burst(h):
            for ic in range(NT):
                et_t = Et[h][ic]
                for jc in range(NT):
                    nc.tensor.matmul(cs[:, jc:jc + 1],
                                     et_t[:, jc * 128:(jc + 1) * 128],
                                     v_sb[:, ic, h, 64:65],
                                     start=(ic == 0 and jc == 0),
                                     stop=(ic == NT - 1 and jc == NT - 1),
                                     skip_group_check=True)

        # rowsum via N=1 matmuls over the E^T slabs, into a borrowed psim tile
        def rs_burst(h):
            rs_t = psim_p.tile([128, 1024], F32, tag="psim", name=f"rs{h}")
            rs = rs_t[:, 0:16]
            for jt in range(NT):
                et_t = ET[h][jt]
                for ic in range(NT):
                    nc.tensor.matmul(rs[:, ic:ic + 1],
                                     et_t[:, ic * 128:(ic + 1) * 128],
                                     mv_sb[:, jt, h, 64:65],
                                     start=(jt == 0 and ic == 0),
                                     stop=(jt == NT - 1 and ic == NT - 1),
                                     skip_group_check=True)
            nc.vector.reciprocal(rrecT[:, h, :], rs)

        def phase_A(h, fillers):
            fi = 0
            for ic in range(NT):
                et_t = e_p.tile([128, N], BF16, tag="et", name=f"et{h}_{ic}")
                Et[h][ic] = et_t
                for half in range(2):
                    ps = psim_p.tile([128, 1024], F32, tag="psim", name="ps")
                    for q in range(2):
                        jn = half * 2 + q
                        nc.tensor.matmul(ps[:, q * 512:(q + 1) * 512],
                                         qkT[h * 64:(h + 1) * 64,
                                             ic * 128:(ic + 1) * 128],
                                         m_qkT[h * 64:(h + 1) * 64,
                                               jn * 512:(jn + 1) * 512],
                                         start=True, stop=True)
                    nc.scalar.activation(et_t[:, half * 1024:(half + 1) * 1024],
                                         ps[:], EXP, scale=SCALE)
                if NA:
                    nc.gpsimd.dma_start(
                        out=Edram[h][ic // 8][(ic % 8) * 128:(ic % 8 + 1) * 128, :],
                        in_=et_t[:, 0:NA * 128])
                while fi <= ic and fi < len(fillers):
                    fillers[fi]()
                    fi += 1
            while fi < len(fillers):
                fillers[fi]()
                fi += 1

        def alpha_grab(h):
            for jt in range(NA):
                et_t = et_a.tile([128, N], BF16, tag="eta", name=f"eta{h}_{jt}")
                ET[h][jt] = et_t

        def alpha_loads(h, ihalf):
            if not ESPLIT and ihalf == 1:
                return
            for jt in range(NA):
                if ESPLIT:
                    # split by i: first half gated only on stores ic<8
                    nc.sync.dma_start_transpose(
                        out=ET[h][jt][:, ihalf * 1024:(ihalf + 1) * 1024],
                        in_=Edram[h][ihalf][:, jt * 128:(jt + 1) * 128])
                else:
                    for hf in range(2):
                        nc.sync.dma_start_transpose(
                            out=ET[h][jt][:, hf * 1024:(hf + 1) * 1024],
                            in_=Edram[h][hf][:, jt * 128:(jt + 1) * 128])

        def gamma_slabs(h):
            for g in range(NG):
                jt = NA + g
                et_t = et_g.tile([128, N], BF16, tag="etg", name=f"etg{h}_{jt}")
                ET[h][jt] = et_t
                for half in range(2):
                    ps = psim_p.tile([128, 1024], F32, tag="psim", name="ps")
                    for q in range(2):
                        ich = half * 2 + q
                        nc.tensor.matmul(ps[:, q * 512:(q + 1) * 512],
                                         m_qkT[h * 64:(h + 1) * 64,
                                               jt * 128:(jt + 1) * 128],
                                         qkT[h * 64:(h + 1) * 64,
                                             ich * 512:(ich + 1) * 512],
                                         start=True, stop=True)
                    nc.scalar.activation(et_t[:, half * 1024:(half + 1) * 1024],
                                         ps[:], EXP, scale=SCALE)

        def colsum_rec(h):
            nc.vector.reciprocal(crecT[:, h, :], cs[:])

        # drain + transpose one side; acc is the psum accumulator being
        # drained, tp_base the (other) psum tensor lending [64, 512] regions
        def drain_transpose(h, acc, rec, dstT, tp_base, tag, act_share=False):
            xn = nrm_p.tile([128, 1024], F32, tag="xn", name=f"xn_{tag}{h}")
            for t in range(NT):
                if act_share and t % 2 == 1:
                    nc.scalar.mul(xn[:, t * 64:(t + 1) * 64],
                                  acc[:, t * 64:(t + 1) * 64],
                                  rec[:, h, t:t + 1])
                else:
                    nc.vector.tensor_scalar_mul(xn[:, t * 64:(t + 1) * 64],
                                                acc[:, t * 64:(t + 1) * 64],
                                                rec[:, h, t:t + 1])
            for b4 in range(4):
                tp = tp_base[0:64, (b4 % 2) * 512:(b4 % 2) * 512 + 512]
                for k in range(4):
                    t = b4 * 4 + k
                    nc.tensor.transpose(tp[:, k * 128:(k + 1) * 128],
                                        xn[:, t * 64:(t + 1) * 64], ident)
                if act_share and b4 % 2 == 1:
                    nc.scalar.mul(dstT[:, h, b4 * 512:(b4 + 1) * 512], tp, 1.0)
                else:
                    nc.vector.tensor_copy(dstT[:, h, b4 * 512:(b4 + 1) * 512], tp)

        def outproj(src, w_sb, dst_dram, pregs, dma_eng, po, act_share=True):
            # copies alternate DVE/Act to halve the serial chain; DMAs go in
            # groups of 4 tiles to amortize HWDGE issue cost
            dst_r = dst_dram.rearrange("(g t p) c -> p g t c", g=4, t=4, p=128)
            nr = len(pregs)
            for t in range(NT):
                reg = pregs[t % nr]
                nc.tensor.matmul(reg, src[:, 0, t * 128:(t + 1) * 128],
                                 w_sb[:, 0, :], start=True, stop=False)
                nc.tensor.matmul(reg, src[:, 1, t * 128:(t + 1) * 128],
                                 w_sb[:, 1, :], start=False, stop=True)
                if act_share and t % 2 == 1:
                    nc.scalar.mul(po[:, t % 8, :], reg, 1.0)
                else:
                    nc.vector.tensor_copy(po[:, t % 8, :], reg)
                if t % 4 == 3:
                    g = t // 4
                    nc_dma = dma_eng
                    nc_dma.dma_start(
                        out=dst_r[:, g, :, :],
                        in_=po[:, (g % 2) * 4:(g % 2) * 4 + 4, :])

        # ================= schedule =================
        # head 0 phase A; m-side + colsum interleaved (x_acc free), with a
        # 2-iteration stagger so chunks never wait on the exp just issued
        fillers0 = [lambda: None, lambda: None] + [
            (lambda ic=ic: m_chunk(0, ic, True)) for ic in range(NT)
        ]
        phase_A(0, fillers0)
        if NA:
            alpha_grab(0)
            alpha_loads(0, 0)   # first i-half gated only on stores ic<8
            alpha_loads(0, 1)
        colsum_rec(0)
        gamma_slabs(0)
        # m-side h0 drain + transposes (borrow x_acc halves for transposes)
        drain_transpose(0, m_acc, crecT, m_outT_b, x_acc, "m")

        # head 1 phase A; fillers: X0 gamma chunks + staggered m1 chunks
        # X0 runs entirely inside A1 as fillers: gamma slabs are resident and
        # the i-split alpha loads of head 0 land early in A1
        xorder = list(range(NA, NT)) + list(range(NA))
        fillers1 = []
        for k in range(NT + 2):
            def f(k=k):
                if k < NT:
                    x_chunk(0, xorder[k], first=(k == 0),
                            last=(k == NT - 1))
                if k >= 2:
                    m_chunk(1, k - 2, False)
            fillers1.append(f)
        phase_A(1, fillers1)
        if NA:
            alpha_grab(1)
            alpha_loads(1, 0)
            alpha_loads(1, 1)
        # rowsum burst gates the h1 alpha loads (pool-slot WAR) and must also
        # precede gamma_slabs(1), whose exps reuse the gamma slabs' pool slots
        rs_burst(0)
        gamma_slabs(1)

        # x-side h0 drain (x_acc frees up), then colsum burst h1 into cs
        xn0 = nrm_p.tile([128, 1024], F32, tag="xn", name="xn_x0")
        for t in range(NT):
            nc.vector.tensor_scalar_mul(xn0[:, t * 64:(t + 1) * 64],
                                        x_acc[:, t * 64:(t + 1) * 64],
                                        rrecT[:, 0, t:t + 1])
        cs_burst(1)
        colsum_rec(1)
        # x0 transposes into x_acc halves (free after cs read)
        for b4 in range(4):
            tp = x_acc[0:64, (b4 % 2) * 512:(b4 % 2) * 512 + 512]
            for k in range(4):
                t = b4 * 4 + k
                nc.tensor.transpose(tp[:, k * 128:(k + 1) * 128],
                                    xn0[:, t * 64:(t + 1) * 64], ident)
            nc.vector.tensor_copy(outT_b[:, 0, b4 * 512:(b4 + 1) * 512], tp)
        # m-side h1 drain + transposes into x_acc halves (x0 drained, X1 not
        # yet started) so pm-outproj's m_acc quarters stay conflict-free
        drain_transpose(1, m_acc, crecT, m_outT_b, x_acc, "m")
        mq = [m_acc[:, i * 256:(i + 1) * 256] for i in range(4)]
        # x-side h1 first: it is Act-gated (gamma exps), so nothing else may
        # sit ahead of it in the in-order PE queue; rowsum matmuls ride along
        # per slab so the reciprocal is ready the moment X1 closes
        rs1_t = psim_p.tile([128, 1024], F32, tag="psim", name="rs1")
        rs1 = rs1_t[:, 0:16]
        for k in range(NT):
            jt = xorder[k]
            x_chunk(1, jt, first=(k == 0), last=(k == NT - 1))
            for ic in range(NT):
                nc.tensor.matmul(rs1[:, ic:ic + 1],
                                 ET[1][jt][:, ic * 128:(ic + 1) * 128],
                                 mv_sb[:, jt, 1, 64:65],
                                 start=(k == 0 and ic == 0),
                                 stop=(k == NT - 1 and ic == NT - 1),
                                 skip_group_check=True)
        nc.vector.reciprocal(rrecT[:, 1, :], rs1)
        # fused tail: per 4-tile group, x-side drain/transpose/project/DMA
        # round-robined with the pm projection groups
        xn1 = nrm_p.tile([128, 1024], F32, tag="xn", name="xn_x1")
        px_r = px.rearrange("(r p) c -> p r c", p=128)
        pm_r = pm.rearrange("(r p) c -> p r c", p=128)
        for b4 in range(4):
            for k in range(4):
                t = b4 * 4 + k
                if k % 2 == 1:
                    nc.scalar.mul(xn1[:, t * 64:(t + 1) * 64],
                                  x_acc[:, t * 64:(t + 1) * 64],
                                  rrecT[:, 1, t:t + 1])
                else:
                    nc.vector.tensor_scalar_mul(xn1[:, t * 64:(t + 1) * 64],
                                                x_acc[:, t * 64:(t + 1) * 64],
                                                rrecT[:, 1, t:t + 1])
            tp_t = psim_p.tile([128, 1024], F32, tag="psim", name=f"tpx{b4}")
            tp = tp_t[0:64, 0:512]
            for k in range(4):
                t = b4 * 4 + k
                nc.tensor.transpose(tp[:, k * 128:(k + 1) * 128],
                                    xn1[:, t * 64:(t + 1) * 64], ident)
            if b4 % 2 == 0:
                nc.vector.tensor_copy(outT_b[:, 1, b4 * 512:(b4 + 1) * 512], tp)
            else:
                nc.scalar.mul(outT_b[:, 1, b4 * 512:(b4 + 1) * 512], tp, 1.0)
            for k in range(4):
                t = b4 * 4 + k
                # bank B of the transpose psim tile: decoupled from pm's mq
                reg = tp_t[:, 512 + (k % 2) * 256: 768 + (k % 2) * 256]
                nc.tensor.matmul(reg, outT_b[:, 0, t * 128:(t + 1) * 128],
                                 wof_sb[:, 0, :], start=True, stop=False)
                nc.tensor.matmul(reg, outT_b[:, 1, t * 128:(t + 1) * 128],
                                 wof_sb[:, 1, :], start=False, stop=True)
                if k % 2 == 0:
                    nc.vector.tensor_copy(po_x[:, t % 8, :], reg)
                else:
                    nc.scalar.mul(po_x[:, t % 8, :], reg, 1.0)
                if k % 2 == 1:
                    nc.sync.dma_start(
                        out=px_r[:, t - 1:t + 1, :],
                        in_=po_x[:, (t - 1) % 8:(t - 1) % 8 + 2, :])
            # pm group b4 rides along
            for k in range(4):
                t = b4 * 4 + k
                reg = mq[2 * (b4 % 2) + k % 2]
                nc.tensor.matmul(reg, m_outT_b[:, 0, t * 128:(t + 1) * 128],
                                 mwof_sb[:, 0, :], start=True, stop=False)
                nc.tensor.matmul(reg, m_outT_b[:, 1, t * 128:(t + 1) * 128],
                                 mwof_sb[:, 1, :], start=False, stop=True)
                if k % 2 == 1:
                    nc.scalar.mul(po_m[:, t % 8, :], reg, 1.0)
                else:
                    nc.vector.tensor_copy(po_m[:, t % 8, :], reg)
                if k % 2 == 1:
                    nc.scalar.dma_start(
                        out=pm_r[:, t - 1:t + 1, :],
                        in_=po_m[:, (t - 1) % 8:(t - 1) % 8 + 2, :])


def kernel(x, m, Wqk, mWqk, Wv, mWv, Wo, bo, mWo, mbo, Wf, bf):
    x = np.asarray(x, dtype=np.float32)
    m = np.asarray(m, dtype=np.float32)
    Wqk = np.asarray(Wqk, dtype=np.float32)
    mWqk = np.asarray(mWqk, dtype=np.float32)
    Wv = np.asarray(Wv, dtype=np.float32)
    mWv = np.asarray(mWv, dtype=np.float32)
    Wo = np.asarray(Wo, dtype=np.float32)
    mWo = np.asarray(mWo, dtype=np.float32)
    Wf = np.asarray(Wf, dtype=np.float32)
    bo = np.asarray(bo, dtype=np.float32)
    mbo = np.asarray(mbo, dtype=np.float32)
    bf = np.asarray(bf, dtype=np.float32)

    if "nc" not in _cache:
        _cache["nc"] = _build()
    nc = _cache["nc"]

    bf16 = ml_dtypes.bfloat16
    wof = (Wo @ Wf).astype(bf16)    # [512, 256]
    mwof = (mWo @ Wf).astype(bf16)
    bias_x = bo @ Wf + bf
    bias_m = mbo @ Wf + bf

    xTs = [np.ascontiguousarray(x[b].T).astype(bf16) for b in range(2)]
    mTs = [np.ascontiguousarray(m[b].T).astype(bf16) for b in range(2)]

    in_maps = []
    for c in range(8):
        b, hp = c // 4, c % 4
        csl = slice(hp * 128, (hp + 1) * 128)
        in_maps.append({
            "xT": xTs[b], "mT": mTs[b],
            "wqk": np.ascontiguousarray(Wqk[:, csl]).astype(bf16),
            "mwqk": np.ascontiguousarray(mWqk[:, csl]).astype(bf16),
            "wv": np.ascontiguousarray(Wv[:, csl]).astype(bf16),
            "mwv": np.ascontiguousarray(mWv[:, csl]).astype(bf16),
            "wof": np.ascontiguousarray(wof[csl, :]),
            "mwof": np.ascontiguousarray(mwof[csl, :]),
        })

    res = run_bass_kernel_spmd(nc, in_maps, list(range(8)))

    out = np.empty((2, 2 * N, DM), dtype=np.float32)
    for b in range(2):
        cores = range(b * 4, b * 4 + 4)
        out[b, :N] = sum(res.results[c]["px"] for c in cores) + bias_x
        out[b, N:] = sum(res.results[c]["pm"] for c in cores) + bias_m
    return out


# revision 76
# speedup vs baseline: 2.3490x; 1.0017x over previous
"""Bidirectional cross-attention kernel for 8 Trainium2 NeuronCores.

Sharding: 16 (batch, head) units across 8 cores -> core c handles
batch b = c//4 and heads (2*(c%4), 2*(c%4)+1).

Per head on each core (all matmul data bf16):
    E   = exp(scale * qk @ m_qk^T)      [i, j] natural, exp on Act engine
          (exp's accum_out gives rowsum[i] for free)
    m-side: m_out[j, d] = sum_i E[i,j] v1[i,d]  via E tiles as stationary
            lhsT (out free = 64 -> half streaming cost); 65th ones column of
            v1 handled by separate N=1 colsum matmuls.
    x-side: needs E^T [j, i]: NA/16 of j-columns round-trip through DRAM and
            the DMA xbar transpose (14 ns per 16x128 tile in the cost model);
            the rest are recomputed as exp(sim^T) on chip.
            out[i, d] = sum_j E^T[j,i]^T mv[j,d] with E^T slabs stationary.
    Normalization = per-partition scalar multiply (DVE tensor_scalar) by
    1/rowsum / 1/colsum during the PSUM->SBUF drain, then PE transposes to
    get out^T for the folded output projection Wof = Wo @ Wf (host-folded).
Host sums the 4 per-batch partials and adds the folded biases.
"""

import numpy as np
import ml_dtypes

import concourse.bass as bass
import concourse.mybir as mybir
import concourse.tile as tile
from concourse import bacc
from concourse.bass_utils import run_bass_kernel_spmd
from concourse.masks import make_identity

F32 = mybir.dt.float32
BF16 = mybir.dt.bfloat16
EXP = mybir.ActivationFunctionType.Exp

import os

N = 2048          # sequence length (i and j)
DM = 256          # model dim
DH = 64           # head dim
NT = N // 128     # 16 row tiles
SCALE = DH ** -0.5
NA0 = int(os.environ.get("BXA_NA0", "7"))  # E^T j-slabs via DRAM xbar, head 0
NA1 = int(os.environ.get("BXA_NA1", "7"))  # same for head 1
NAs = [NA0, NA1]
NGs = [NT - NA0, NT - NA1]

_cache = {}


def _build():
    nc = bacc.Bacc("TRN2", target_bir_lowering=False, debug=False, num_devices=8)

    xT = nc.dram_tensor("xT", [DM, N], BF16, kind="ExternalInput")
    mT = nc.dram_tensor("mT", [DM, N], BF16, kind="ExternalInput")
    wqk = nc.dram_tensor("wqk", [DM, 128], BF16, kind="ExternalInput")
    mwqk = nc.dram_tensor("mwqk", [DM, 128], BF16, kind="ExternalInput")
    wv = nc.dram_tensor("wv", [DM, 128], BF16, kind="ExternalInput")
    mwv = nc.dram_tensor("mwv", [DM, 128], BF16, kind="ExternalInput")
    wof = nc.dram_tensor("wof", [128, DM], BF16, kind="ExternalInput")
    mwof = nc.dram_tensor("mwof", [128, DM], BF16, kind="ExternalInput")
    px = nc.dram_tensor("px", [N, DM], F32, kind="ExternalOutput")
    pm = nc.dram_tensor("pm", [N, DM], F32, kind="ExternalOutput")

    with tile.TileContext(nc) as tc:
        _emit(tc, xT, mT, wqk, mwqk, wv, mwv, wof, mwof, px, pm)
    nc.compile()
    return nc


def _emit(tc, xT, mT, wqk, mwqk, wv, mwv, wof, mwof, px, pm):
    nc = tc.nc
    import contextlib
    ctx = contextlib.ExitStack()
    with ctx:
        singles = ctx.enter_context(tc.tile_pool(name="singles", bufs=1))
        e_p = ctx.enter_context(tc.tile_pool(name="e", bufs=16))
        et_a = ctx.enter_context(tc.tile_pool(name="eta", bufs=max(NA0, NA1, 1)))
        et_g = ctx.enter_context(tc.tile_pool(name="etg", bufs=max(NT - NA0, NT - NA1, 1)))
        nrm_p = ctx.enter_context(tc.tile_pool(name="nrm", bufs=2))
        psim_p = ctx.enter_context(tc.tile_pool(name="psim", bufs=2, space="PSUM"))
        pacc_p = ctx.enter_context(tc.tile_pool(name="pacc", bufs=1, space="PSUM"))

        ident = singles.tile([128, 128], F32)
        make_identity(nc, ident)

        # ---- weights / inputs (qk weights + mT + xT first: they gate A0) ----
        wqk_sb = singles.tile([128, 2, 128], BF16)
        mwqk_sb = singles.tile([128, 2, 128], BF16)
        wv_sb = singles.tile([128, 2, 128], BF16)
        mwv_sb = singles.tile([128, 2, 128], BF16)
        for t, d in ((wqk_sb, wqk), (mwqk_sb, mwqk)):
            nc.sync.dma_start(out=t[:], in_=d.rearrange("(k p) n -> p k n", p=128))
        xT_sb = singles.tile([128, 2, N], BF16)
        mT_sb = singles.tile([128, 2, N], BF16)
        for src, dst, q in ((mT, mT_sb, 0), (xT, xT_sb, 0), (mT, mT_sb, 1),
                            (mT, mT_sb, 2), (mT, mT_sb, 3), (xT, xT_sb, 1),
                            (xT, xT_sb, 2), (xT, xT_sb, 3)):
            nc.sync.dma_start(
                out=dst[:, :, q * 512:(q + 1) * 512],
                in_=src.rearrange("(k p) n -> p k n", p=128)[:, :, q * 512:(q + 1) * 512])
        for t, d in ((wv_sb, wv), (mwv_sb, mwv)):
            nc.sync.dma_start(out=t[:], in_=d.rearrange("(k p) n -> p k n", p=128))
        wof_sb = singles.tile([64, 2, DM], BF16)
        mwof_sb = singles.tile([64, 2, DM], BF16)
        nc.sync.dma_start(out=wof_sb[:], in_=wof.rearrange("(h d) n -> d h n", d=64))
        nc.sync.dma_start(out=mwof_sb[:], in_=mwof.rearrange("(h d) n -> d h n", d=64))

        qkT = singles.tile([128, N], BF16)
        m_qkT = singles.tile([128, N], BF16)
        v_sb = singles.tile([128, NT, 2, 65], BF16)
        mv_sb = singles.tile([128, NT, 2, 65], BF16)
        rrecT = singles.tile([128, 2, NT], F32)       # 1/rowsum, i on partitions
        crecT = singles.tile([128, 2, NT], F32)       # 1/colsum, j on partitions
        outT_b = singles.tile([64, 2, N], BF16)       # x-side out^T
        m_outT_b = singles.tile([64, 2, N], BF16)     # m-side out^T
        po_m = singles.tile([128, 8, DM], F32)        # pm staging
        po_x = singles.tile([128, 8, DM], F32)        # px staging

        m_acc = pacc_p.tile([128, 1024], F32, tag="macc")   # banks 4-5
        x_acc = pacc_p.tile([128, 1024], F32, tag="xacc")   # banks 6-7
        cs = x_acc[:, 0:16]                                 # colsum sliver

        ESPLIT = int(os.environ.get("BXA_ESPLIT", "1"))
        Edram = [[nc.dram_tensor(f"edram{h}_{hf}", [N // 2, NAs[h] * 128],
                                 BF16).ap()
                  for hf in range(2)] if NAs[h] else None for h in range(2)]

        # PE p-state warmup: dummy transposes while the input DMAs stream in,
        # so the projections and first sims run at full clock
        for _ in range(14):
            nc.tensor.transpose(x_acc[:, 768:896], ident, ident)

        # ---- projections (psum borrowed from m_acc / x_acc halves) ----
        qk_jobs = [(mwqk_sb, mT_sb, m_qkT, m_acc, 0), (wqk_sb, xT_sb, qkT, x_acc, 0),
                   (mwqk_sb, mT_sb, m_qkT, m_acc, 1), (mwqk_sb, mT_sb, m_qkT, m_acc, 2),
                   (mwqk_sb, mT_sb, m_qkT, m_acc, 3), (wqk_sb, xT_sb, qkT, x_acc, 1),
                   (wqk_sb, xT_sb, qkT, x_acc, 2), (wqk_sb, xT_sb, qkT, x_acc, 3)]
        for w_sb, src, dst, pb, t in qk_jobs:
            pq = pb[:, (t % 2) * 512:(t % 2) * 512 + 512]
            for kc in range(2):
                nc.tensor.matmul(pq, w_sb[:, kc, :],
                                 src[:, kc, t * 512:(t + 1) * 512],
                                 start=(kc == 0), stop=(kc == 1))
            nc.vector.tensor_copy(dst[:, t * 512:(t + 1) * 512], pq)
        for wi, (w_sb, src, dst) in enumerate(
                ((wv_sb, xT_sb, v_sb), (mwv_sb, mT_sb, mv_sb))):
            for g in range(4):
                k4 = (wi * 4 + g) % 4
                pv = (m_acc if k4 < 2 else x_acc)[:, (k4 % 2) * 512:
                                                  (k4 % 2) * 512 + 512]
                for dt_ in range(4):
                    t = g * 4 + dt_
                    for kc in range(2):
                        nc.tensor.matmul(pv[:, dt_ * 128:(dt_ + 1) * 128],
                                         src[:, kc, t * 128:(t + 1) * 128],
                                         w_sb[:, kc, :],
                                         start=(kc == 0), stop=(kc == 1))
                nc.vector.tensor_copy(
                    dst[:, g * 4:(g + 1) * 4, :, 0:64],
                    pv.rearrange("p (t h d) -> p t h d", t=4, h=2))
            nc.vector.memset(dst[:, :, :, 64:65], 1.0)

        Et = [[None] * NT, [None] * NT]
        ET = [[None] * NT, [None] * NT]

        # PSUM start=True zeroes the WHOLE 2KB bank (zero region), so a bank
        # shared by interleaved accumulation chains must see exactly ONE
        # start (its very first matmul) and ONE stop (its very last).
        # m-side chunk: for one ic, accumulate all 16 j-tiles (+ colsum for
        # h=0, where x_acc is free during phase A)
        def m_chunk(h, ic, with_cs):
            et_t = Et[h][ic]
            for jc in range(NT):
                nc.tensor.matmul(m_acc[:, jc * 64:(jc + 1) * 64],
                                 et_t[:, jc * 128:(jc + 1) * 128],
                                 v_sb[:, ic, h, 0:64],
                                 start=(ic == 0 and jc % 8 == 0),
                                 stop=(ic == NT - 1 and jc % 8 == 7),
                                 skip_group_check=True)
                if with_cs:
                    nc.tensor.matmul(cs[:, jc:jc + 1],
                                     et_t[:, jc * 128:(jc + 1) * 128],
                                     v_sb[:, ic, h, 64:65],
                                     start=(ic == 0 and jc == 0),
                                     stop=(ic == NT - 1 and jc == NT - 1),
                                     skip_group_check=True)

        # x-side chunk: one j-slab of E^T against all 16 i-tiles
        def x_chunk(h, jt, first, last):
            et_t = ET[h][jt]
            for ic in range(NT):
                nc.tensor.matmul(x_acc[:, ic * 64:(ic + 1) * 64],
                                 et_t[:, ic * 128:(ic + 1) * 128],
                                 mv_sb[:, jt, h, 0:64],
                                 start=(first and ic % 8 == 0),
                                 stop=(last and ic % 8 == 7),
                                 skip_group_check=True)

        def cs_burst(h):
            for ic in range(NT):
                et_t = Et[h][ic]
                for jc in range(NT):
                    nc.tensor.matmul(cs[:, jc:jc + 1],
                                     et_t[:, jc * 128:(jc + 1) * 128],
                                     v_sb[:, ic, h, 64:65],
                                     start=(ic == 0 and jc == 0),
                                     stop=(ic == NT - 1 and jc == NT - 1),
                                     skip_group_check=True)

        # rowsum via N=1 matmuls over the E^T slabs, into a borrowed psim tile
        def rs_burst(h):
            rs_t = psim_p.tile([128, 1024], F32, tag="psim", name=f"rs{h}")
            rs = rs_t[:, 0:16]
            for jt in range(NT):
                et_t = ET[h][jt]
                for ic in range(NT):
                    nc.tensor.matmul(rs[:, ic:ic + 1],
                                     et_t[:, ic * 128:(ic + 1) * 128],
                                     mv_sb[:, jt, h, 64:65],
                                     start=(jt == 0 and ic == 0),
                                     stop=(jt == NT - 1 and ic == NT - 1),
                                     skip_group_check=True)
            nc.vector.reciprocal(rrecT[:, h, :], rs)

        def phase_A(h, fillers):
            fi = 0
            for ic in range(NT):
                et_t = e_p.tile([128, N], BF16, tag="et", name=f"et{h}_{ic}")
                Et[h][ic] = et_t
                for half in range(2):
                    ps = psim_p.tile([128, 1024], F32, tag="psim", name="ps")
                    for q in range(2):
                        jn = half * 2 + q
                        nc.tensor.matmul(ps[:, q * 512:(q + 1) * 512],
                                         qkT[h * 64:(h + 1) * 64,
                                             ic * 128:(ic + 1) * 128],
                                         m_qkT[h * 64:(h + 1) * 64,
                                               jn * 512:(jn + 1) * 512],
                                         start=True, stop=True)
                    nc.scalar.activation(et_t[:, half * 1024:(half + 1) * 1024],
                                         ps[:], EXP, scale=SCALE)
                if NAs[h]:
                    nc.gpsimd.dma_start(
                        out=Edram[h][ic // 8][(ic % 8) * 128:(ic % 8 + 1) * 128, :],
                        in_=et_t[:, 0:NAs[h] * 128])
                while fi <= ic and fi < len(fillers):
                    fillers[fi]()
                    fi += 1
            while fi < len(fillers):
                fillers[fi]()
                fi += 1

        def alpha_grab(h):
            for jt in range(NAs[h]):
                et_t = et_a.tile([128, N], BF16, tag="eta", name=f"eta{h}_{jt}")
                ET[h][jt] = et_t

        def alpha_loads(h, ihalf):
            if not ESPLIT and ihalf == 1:
                return
            for jt in range(NAs[h]):
                if ESPLIT:
                    # split by i: first half gated only on stores ic<8
                    nc.sync.dma_start_transpose(
                        out=ET[h][jt][:, ihalf * 1024:(ihalf + 1) * 1024],
                        in_=Edram[h][ihalf][:, jt * 128:(jt + 1) * 128])
                else:
                    for hf in range(2):
                        nc.sync.dma_start_transpose(
                            out=ET[h][jt][:, hf * 1024:(hf + 1) * 1024],
                            in_=Edram[h][hf][:, jt * 128:(jt + 1) * 128])

        def gamma_slabs(h):
            for g in range(NGs[h]):
                jt = NAs[h] + g
                et_t = et_g.tile([128, N], BF16, tag="etg", name=f"etg{h}_{jt}")
                ET[h][jt] = et_t
                for half in range(2):
                    ps = psim_p.tile([128, 1024], F32, tag="psim", name="ps")
                    for q in range(2):
                        ich = half * 2 + q
                        nc.tensor.matmul(ps[:, q * 512:(q + 1) * 512],
                                         m_qkT[h * 64:(h + 1) * 64,
                                               jt * 128:(jt + 1) * 128],
                                         qkT[h * 64:(h + 1) * 64,
                                             ich * 512:(ich + 1) * 512],
                                         start=True, stop=True)
                    nc.scalar.activation(et_t[:, half * 1024:(half + 1) * 1024],
                                         ps[:], EXP, scale=SCALE)

        def colsum_rec(h):
            nc.vector.reciprocal(crecT[:, h, :], cs[:])

        # drain + transpose one side; acc is the psum accumulator being
        # drained, tp_base the (other) psum tensor lending [64, 512] regions
        def drain_transpose(h, acc, rec, dstT, tp_base, tag, act_share=False):
            xn = nrm_p.tile([128, 1024], F32, tag="xn", name=f"xn_{tag}{h}")
            for t in range(NT):
                if act_share and t % 2 == 1:
                    nc.scalar.mul(xn[:, t * 64:(t + 1) * 64],
                                  acc[:, t * 64:(t + 1) * 64],
                                  rec[:, h, t:t + 1])
                else:
                    nc.vector.tensor_scalar_mul(xn[:, t * 64:(t + 1) * 64],
                                                acc[:, t * 64:(t + 1) * 64],
                                                rec[:, h, t:t + 1])
            for b4 in range(4):
                tp = tp_base[0:64, (b4 % 2) * 512:(b4 % 2) * 512 + 512]
                for k in range(4):
                    t = b4 * 4 + k
                    nc.tensor.transpose(tp[:, k * 128:(k + 1) * 128],
                                        xn[:, t * 64:(t + 1) * 64], ident)
                if act_share and b4 % 2 == 1:
                    nc.scalar.mul(dstT[:, h, b4 * 512:(b4 + 1) * 512], tp, 1.0)
                else:
                    nc.vector.tensor_copy(dstT[:, h, b4 * 512:(b4 + 1) * 512], tp)

        def outproj(src, w_sb, dst_dram, pregs, dma_eng, po, act_share=True):
            # copies alternate DVE/Act to halve the serial chain; DMAs go in
            # groups of 4 tiles to amortize HWDGE issue cost
            dst_r = dst_dram.rearrange("(g t p) c -> p g t c", g=4, t=4, p=128)
            nr = len(pregs)
            for t in range(NT):
                reg = pregs[t % nr]
                nc.tensor.matmul(reg, src[:, 0, t * 128:(t + 1) * 128],
                                 w_sb[:, 0, :], start=True, stop=False)
                nc.tensor.matmul(reg, src[:, 1, t * 128:(t + 1) * 128],
                                 w_sb[:, 1, :], start=False, stop=True)
                if act_share and t % 2 == 1:
                    nc.scalar.mul(po[:, t % 8, :], reg, 1.0)
                else:
                    nc.vector.tensor_copy(po[:, t % 8, :], reg)
                if t % 4 == 3:
                    g = t // 4
                    nc_dma = dma_eng
                    nc_dma.dma_start(
                        out=dst_r[:, g, :, :],
                        in_=po[:, (g % 2) * 4:(g % 2) * 4 + 4, :])

        # ================= schedule =================
        # head 0 phase A; m-side + colsum interleaved (x_acc free), with a
        # 2-iteration stagger so chunks never wait on the exp just issued
        fillers0 = [lambda: None, lambda: None] + [
            (lambda ic=ic: m_chunk(0, ic, True)) for ic in range(NT)
        ]
        phase_A(0, fillers0)
        if NA0:
            alpha_grab(0)
            alpha_loads(0, 0)   # first i-half gated only on stores ic<8
            alpha_loads(0, 1)
        colsum_rec(0)
        gamma_slabs(0)
        # m-side h0 drain + transposes (borrow x_acc halves for transposes)
        drain_transpose(0, m_acc, crecT, m_outT_b, x_acc, "m")

        # head 1 phase A; fillers: X0 gamma chunks + staggered m1 chunks
        # X0 runs entirely inside A1 as fillers: gamma slabs are resident and
        # the i-split alpha loads of head 0 land early in A1
        xorder = list(range(NA0, NT)) + list(range(NA0))
        xorder1 = list(range(NA1)) + list(range(NA1, NT))
        fillers1 = []
        for k in range(NT + 2):
            def f(k=k):
                if k < NT:
                    x_chunk(0, xorder[k], first=(k == 0),
                            last=(k == NT - 1))
                if k >= 2:
                    m_chunk(1, k - 2, False)
            fillers1.append(f)
        phase_A(1, fillers1)
        if NA1:
            alpha_grab(1)
            alpha_loads(1, 0)
            alpha_loads(1, 1)
        # rowsum burst gates the h1 alpha loads (pool-slot WAR) and must also
        # precede gamma_slabs(1), whose exps reuse the gamma slabs' pool slots
        rs_burst(0)
        gamma_slabs(1)

        # x-side h0 drain (x_acc frees up), then colsum burst h1 into cs
        xn0 = nrm_p.tile([128, 1024], F32, tag="xn", name="xn_x0")
        for t in range(NT):
            nc.vector.tensor_scalar_mul(xn0[:, t * 64:(t + 1) * 64],
                                        x_acc[:, t * 64:(t + 1) * 64],
                                        rrecT[:, 0, t:t + 1])
        cs_burst(1)
        colsum_rec(1)
        # x0 transposes into x_acc halves (free after cs read)
        for b4 in range(4):
            tp = x_acc[0:64, (b4 % 2) * 512:(b4 % 2) * 512 + 512]
            for k in range(4):
                t = b4 * 4 + k
                nc.tensor.transpose(tp[:, k * 128:(k + 1) * 128],
                                    xn0[:, t * 64:(t + 1) * 64], ident)
            nc.vector.tensor_copy(outT_b[:, 0, b4 * 512:(b4 + 1) * 512], tp)
        # m-side h1 drain + transposes into x_acc halves (x0 drained, X1 not
        # yet started) so pm-outproj's m_acc quarters stay conflict-free
        drain_transpose(1, m_acc, crecT, m_outT_b, m_acc, "m")
        mq = [m_acc[:, i * 256:(i + 1) * 256] for i in range(4)]
        # x-side h1 first: it is Act-gated (gamma exps), so nothing else may
        # sit ahead of it in the in-order PE queue; rowsum matmuls ride along
        # per slab so the reciprocal is ready the moment X1 closes
        rs1_t = psim_p.tile([128, 1024], F32, tag="psim", name="rs1")
        rs1 = rs1_t[:, 0:16]
        for k in range(NT):
            jt = xorder1[k]
            x_chunk(1, jt, first=(k == 0), last=(k == NT - 1))
            for ic in range(NT):
                nc.tensor.matmul(rs1[:, ic:ic + 1],
                                 ET[1][jt][:, ic * 128:(ic + 1) * 128],
                                 mv_sb[:, jt, 1, 64:65],
                                 start=(k == 0 and ic == 0),
                                 stop=(k == NT - 1 and ic == NT - 1),
                                 skip_group_check=True)
        nc.vector.reciprocal(rrecT[:, 1, :], rs1)
        # fused tail: per 4-tile group, x-side drain/transpose/project/DMA
        # round-robined with the pm projection groups
        xn1 = nrm_p.tile([128, 1024], F32, tag="xn", name="xn_x1")
        px_r = px.rearrange("(r p) c -> p r c", p=128)
        pm_r = pm.rearrange("(r p) c -> p r c", p=128)
        for b4 in range(4):
            for k in range(4):
                t = b4 * 4 + k
                if k % 2 == 1:
                    nc.scalar.mul(xn1[:, t * 64:(t + 1) * 64],
                                  x_acc[:, t * 64:(t + 1) * 64],
                                  rrecT[:, 1, t:t + 1])
                else:
                    nc.vector.tensor_scalar_mul(xn1[:, t * 64:(t + 1) * 64],
                                                x_acc[:, t * 64:(t + 1) * 64],
                                                rrecT[:, 1, t:t + 1])
            tp_t = psim_p.tile([128, 1024], F32, tag="psim", name=f"tpx{b4}")
            tp = tp_t[0:64, 0:512]
            for k in range(4):
                t = b4 * 4 + k
                nc.tensor.transpose(tp[:, k * 128:(k + 1) * 128],
                                    xn1[:, t * 64:(t + 1) * 64], ident)
            if b4 % 2 == 0:
                nc.vector.tensor_copy(outT_b[:, 1, b4 * 512:(b4 + 1) * 512], tp)
            else:
                nc.scalar.mul(outT_b[:, 1, b4 * 512:(b4 + 1) * 512], tp, 1.0)
            for k in range(4):
                t = b4 * 4 + k
                # bank B of the transpose psim tile: decoupled from pm's mq
                reg = tp_t[:, 512 + (k % 2) * 256: 768 + (k % 2) * 256]
                nc.tensor.matmul(reg, outT_b[:, 0, t * 128:(t + 1) * 128],
                                 wof_sb[:, 0, :], start=True, stop=False)
                nc.tensor.matmul(reg, outT_b[:, 1, t * 128:(t + 1) * 128],
                                 wof_sb[:, 1, :], start=False, stop=True)
                if k % 2 == 0:
                    nc.vector.tensor_copy(po_x[:, t % 8, :], reg)
                else:
                    nc.scalar.mul(po_x[:, t % 8, :], reg, 1.0)
                if k % 2 == 1:
                    nc.sync.dma_start(
                        out=px_r[:, t - 1:t + 1, :],
                        in_=po_x[:, (t - 1) % 8:(t - 1) % 8 + 2, :])
            # pm group b4 rides along
            for k in range(4):
                t = b4 * 4 + k
                reg = mq[2 * (b4 % 2) + k % 2]
                nc.tensor.matmul(reg, m_outT_b[:, 0, t * 128:(t + 1) * 128],
                                 mwof_sb[:, 0, :], start=True, stop=False)
                nc.tensor.matmul(reg, m_outT_b[:, 1, t * 128:(t + 1) * 128],
                                 mwof_sb[:, 1, :], start=False, stop=True)
                if k % 2 == 1:
                    nc.scalar.mul(po_m[:, t % 8, :], reg, 1.0)
                else:
                    nc.vector.tensor_copy(po_m[:, t % 8, :], reg)
                if k % 2 == 1:
                    nc.scalar.dma_start(
                        out=pm_r[:, t - 1:t + 1, :],
                        in_=po_m[:, (t - 1) % 8:(t - 1) % 8 + 2, :])


def kernel(x, m, Wqk, mWqk, Wv, mWv, Wo, bo, mWo, mbo, Wf, bf):
    x = np.asarray(x, dtype=np.float32)
    m = np.asarray(m, dtype=np.float32)
    Wqk = np.asarray(Wqk, dtype=np.float32)
    mWqk = np.asarray(mWqk, dtype=np.float32)
    Wv = np.asarray(Wv, dtype=np.float32)
    mWv = np.asarray(mWv, dtype=np.float32)
    Wo = np.asarray(Wo, dtype=np.float32)
    mWo = np.asarray(mWo, dtype=np.float32)
    Wf = np.asarray(Wf, dtype=np.float32)
    bo = np.asarray(bo, dtype=np.float32)
    mbo = np.asarray(mbo, dtype=np.float32)
    bf = np.asarray(bf, dtype=np.float32)

    if "nc" not in _cache:
        _cache["nc"] = _build()
    nc = _cache["nc"]

    bf16 = ml_dtypes.bfloat16
    wof = (Wo @ Wf).astype(bf16)    # [512, 256]
    mwof = (mWo @ Wf).astype(bf16)
    bias_x = bo @ Wf + bf
    bias_m = mbo @ Wf + bf

    xTs = [np.ascontiguousarray(x[b].T).astype(bf16) for b in range(2)]
    mTs = [np.ascontiguousarray(m[b].T).astype(bf16) for b in range(2)]

    in_maps = []
    for c in range(8):
        b, hp = c // 4, c % 4
        csl = slice(hp * 128, (hp + 1) * 128)
        in_maps.append({
            "xT": xTs[b], "mT": mTs[b],
            "wqk": np.ascontiguousarray(Wqk[:, csl]).astype(bf16),
            "mwqk": np.ascontiguousarray(mWqk[:, csl]).astype(bf16),
            "wv": np.ascontiguousarray(Wv[:, csl]).astype(bf16),
            "mwv": np.ascontiguousarray(mWv[:, csl]).astype(bf16),
            "wof": np.ascontiguousarray(wof[csl, :]),
            "mwof": np.ascontiguousarray(mwof[csl, :]),
        })

    res = run_bass_kernel_spmd(nc, in_maps, list(range(8)))

    out = np.empty((2, 2 * N, DM), dtype=np.float32)
    for b in range(2):
        cores = range(b * 4, b * 4 + 4)
        out[b, :N] = sum(res.results[c]["px"] for c in cores) + bias_x
        out[b, N:] = sum(res.results[c]["pm"] for c in cores) + bias_m
    return out
